# revision 13
# baseline (speedup 1.0000x reference)
"""DeepSNN Trainium2 kernel: 2-layer LIF SNN over 20 timesteps.

Strategy (pure data parallel over 8 cores, 2048 batch rows each):
- Host pre-transposes x -> xT [1700, 2048] per core so the contraction dim
  lands on SBUF partitions with fully contiguous DMAs (no on-device transpose).
- All matmuls run in fp32r (the PE's full-rate ~12-bit-mantissa fp32 mode);
  operands are pre-rounded host-side so DMAs stay pure byte copies, and the
  on-device state writes round on write via out-AP dtype.
- fp32r matmuls require dst PSUM base partition 0, so the big matmul packs two
  batch groups per column via a K=64 block-diagonal W1 (M=128), and the cur2
  matmuls write two column ranges of one [64, 512] PSUM bank.
- Membrane state is stored NEGATED (M = -mem) so one LIF update is exactly two
  fused scalar_tensor_tensor ops:
      t  = (M * beta) - cur          (POOL engine)
      M' = (M < -1) + t              (DVE engine)
- cur2 + beta*M2 - b2 accumulate fully in PSUM (beta*I64, block-diag W2, K=1
  bias matmuls), so the mem2 update is one DVE pass reading PSUM.
- Batch is split into 2 halves per core to pipeline DMA / big matmul /
  recurrent loop; half-1's big matmul interleaves into half-0's loop on PE.
"""

import sys

if "/opt/trn_rl_repo" not in sys.path:
    sys.path.insert(0, "/opt/trn_rl_repo")

import numpy as np

B_TOTAL = 16384
N_CORES = 8
BC = 2048  # batch per core
KDIM = 1700
NKCH = 27  # K chunks of 64 (26*64 + 36)
KROWS = [64] * 26 + [36]
HB = 1024  # batch per half
W = 512  # M1 free width per half
NSTEP = 20
BETA = 0.9

_CACHE = {}


def _emit_bigmm_chunk(nc, mybir, sb, xt_d, w1blk, psum_c1, h, c):
    f32r = mybir.dt.float32r
    kr = KROWS[c]
    kr2 = 2 * kr
    xk = sb.tile([128, W], f32r, tag="xk", bufs=6, name=f"xk_{h}_{c}")
    src = xt_d[64 * c : 64 * c + kr, HB * h : HB * h + HB].rearrange(
        "k (b n) -> b k n", b=2
    )
    nc.sync.dma_start(xk[0:kr2, :], src)
    nc.tensor.matmul(
        psum_c1[:, :],
        w1blk[0:kr2, 128 * c : 128 * c + 128],
        xk[0:kr2, :],
        start=False,
        stop=False,
    )


def _emit_step(nc, mybir, sb, ps, h, t, M1, M2, cur1, consts):
    f32 = mybir.dt.float32
    f32r = mybir.dt.float32r
    ALU = mybir.AluOpType
    w2p, b2qn, bI64, ones = consts

    u = sb.tile([128, W], f32, tag=f"u_{h}", bufs=2, name=f"u_{h}_{t}")
    nc.scalar.mul(u[:], M1[:], BETA)
    t1 = sb.tile([128, W], f32, tag=f"t1_{h}", bufs=2, name=f"t1_{h}_{t}")
    nc.gpsimd.tensor_add(t1[:], u[:], cur1[:])
    M1n = sb.tile([128, W], f32, tag=f"m1_{h}", bufs=2, name=f"m1_{h}_{t}")
    nc.vector.scalar_tensor_tensor(
        M1n[:].bitcast(f32r), M1[:], -1.0, t1[:], ALU.is_lt, ALU.add
    )
    pt2 = ps.tile([64, 512], f32, tag=f"t2_{h}", bufs=2, name=f"pt2_{h}_{t}")
    nc.tensor.matmul(
        pt2[:, :], bI64[:], M2[:].bitcast(f32r), start=True, stop=False
    )
    nc.tensor.matmul(
        pt2[:, 0:256], w2p[:], M1n[:, 0:256].bitcast(f32r),
        start=False, stop=False,
    )
    nc.tensor.matmul(
        pt2[:, 256:512], w2p[:], M1n[:, 256:512].bitcast(f32r),
        start=False, stop=False,
    )
    nc.tensor.matmul(
        pt2[:, :], b2qn[0:1, :], ones[0:1, :], start=False, stop=True
    )
    M2n = sb.tile([64, 512], f32, tag=f"m2_{h}", bufs=2, name=f"m2_{h}_{t}")
    nc.vector.scalar_tensor_tensor(
        M2n[:].bitcast(f32r), M2[:], -1.0, pt2[:, :], ALU.is_lt, ALU.add
    )
    return M1n, M2n


def _emit_finals(nc, mybir, psum_out, M2, w3r, h):
    f32r = mybir.dt.float32r
    for a in range(2):
        for b in range(2):
            for c in range(2):
                s = 8 * h + 4 * a + 2 * b + c
                first = h == 0 and a == 0 and b == 0 and c == 0
                last = h == 1 and a == 1 and b == 1 and c == 1
                off = 256 * b + 128 * c
                nc.tensor.matmul(
                    psum_out[:, 2 * s : 2 * s + 2],
                    M2[32 * a : 32 * a + 32, off : off + 128].bitcast(f32r),
                    w3r[32 * a : 32 * a + 32, :],
                    start=first,
                    stop=last,
                    tile_position=(32 * a, 0),
                )


def build_nc():
    import concourse.tile as tile
    from concourse import bacc, mybir

    f32 = mybir.dt.float32
    f32r = mybir.dt.float32r

    nc = bacc.Bacc("TRN2", target_bir_lowering=False, debug=False)
    xt_d = nc.dram_tensor("xt", [KDIM, BC], f32r, kind="ExternalInput")
    w1blk_d = nc.dram_tensor("w1blk", [128, NKCH * 128], f32r, kind="ExternalInput")
    b1q_d = nc.dram_tensor("b1q", [1, 128], f32r, kind="ExternalInput")
    w2p_d = nc.dram_tensor("w2p", [128, 64], f32r, kind="ExternalInput")
    b2qn_d = nc.dram_tensor("b2qn", [1, 64], f32r, kind="ExternalInput")
    bI64_d = nc.dram_tensor("bI64", [64, 64], f32r, kind="ExternalInput")
    w3r_d = nc.dram_tensor("w3r", [64, 2], f32r, kind="ExternalInput")
    b3bc_d = nc.dram_tensor("b3bc", [128, 32], f32, kind="ExternalInput")
    ones_d = nc.dram_tensor("ones", [1, 512], f32r, kind="ExternalInput")
    zrow_d = nc.dram_tensor("zrow", [1, 128], f32r, kind="ExternalInput")
    out_d = nc.dram_tensor("out", [128, 32], f32, kind="ExternalOutput")

    with tile.TileContext(nc) as tc:
        with (
            tc.tile_pool(name="sb", bufs=1) as sb,
            tc.tile_pool(name="ps", bufs=1, space="PSUM") as ps,
        ):
            # ---- constants ----
            w1blk = sb.tile([128, NKCH * 128], f32r, name="w1blk_sb")
            nc.sync.dma_start(w1blk[:], w1blk_d[:])
            b1q = sb.tile([1, 128], f32r, name="b1q_sb")
            nc.sync.dma_start(b1q[:], b1q_d[:])
            w2p = sb.tile([128, 64], f32r, name="w2p_sb")
            nc.sync.dma_start(w2p[:], w2p_d[:])
            b2qn = sb.tile([1, 64], f32r, name="b2qn_sb")
            nc.sync.dma_start(b2qn[:], b2qn_d[:])
            bI64 = sb.tile([64, 64], f32r, name="bI64_sb")
            nc.sync.dma_start(bI64[:], bI64_d[:])
            w3r = sb.tile([64, 2], f32r, name="w3r_sb")
            nc.sync.dma_start(w3r[:], w3r_d[:])
            b3bc = sb.tile([128, 32], f32, name="b3bc_sb")
            nc.sync.dma_start(b3bc[:], b3bc_d[:])
            ones = sb.tile([1, 512], f32r, name="ones_sb")
            nc.sync.dma_start(ones[:], ones_d[:])
            zrow = sb.tile([1, 128], f32r, name="zrow_sb")
            nc.sync.dma_start(zrow[:], zrow_d[:])
            consts = (w2p, b2qn, bI64, ones)

            psum_out = ps.tile([128, 512], f32, tag="out", bufs=1, name="psum_out")

            # ---- half 0: DMA + big matmul ----
            pc1_0 = ps.tile([128, 512], f32, tag="c1_0", bufs=1, name="pc1_0")
            nc.tensor.matmul(
                pc1_0[:, :], zrow[0:1, :], ones[0:1, :], start=True, stop=False
            )
            for c in range(NKCH):
                _emit_bigmm_chunk(nc, mybir, sb, xt_d, w1blk, pc1_0, 0, c)
            nc.tensor.matmul(
                pc1_0[:, :], b1q[0:1, :], ones[0:1, :], start=False, stop=True
            )
            cur1_0 = sb.tile([128, W], f32, name="cur1_0")
            nc.scalar.copy(cur1_0[:], pc1_0[:, :])

            # half-1 cur1 psum opened early so its chunks can slot in
            pc1_1 = ps.tile([128, 512], f32, tag="c1_1", bufs=1, name="pc1_1")
            nc.tensor.matmul(
                pc1_1[:, :], zrow[0:1, :], ones[0:1, :], start=True, stop=False
            )

            # ---- half 0 recurrent loop (half-1 big-mm interleaved on PE) ----
            M1 = sb.tile([128, W], f32, tag="m1_0", bufs=2, name="m1_0_init")
            nc.vector.memset(M1[:], 0.0)
            M2 = sb.tile([64, 512], f32, tag="m2_0", bufs=2, name="m2_0_init")
            nc.gpsimd.memset(M2[:], 0.0)
            for t in range(NSTEP):
                M1, M2 = _emit_step(nc, mybir, sb, ps, 0, t, M1, M2, cur1_0, consts)
                if t < 14:
                    _emit_bigmm_chunk(nc, mybir, sb, xt_d, w1blk, pc1_1, 1, 2 * t)
                    if 2 * t + 1 < NKCH:
                        _emit_bigmm_chunk(
                            nc, mybir, sb, xt_d, w1blk, pc1_1, 1, 2 * t + 1
                        )
            _emit_finals(nc, mybir, psum_out, M2, w3r, 0)

            # ---- half 1 ----
            nc.tensor.matmul(
                pc1_1[:, :], b1q[0:1, :], ones[0:1, :], start=False, stop=True
            )
            cur1_1 = sb.tile([128, W], f32, name="cur1_1")
            nc.scalar.copy(cur1_1[:], pc1_1[:, :])
            M1 = sb.tile([128, W], f32, tag="m1_1", bufs=2, name="m1_1_init")
            nc.vector.memset(M1[:], 0.0)
            M2 = sb.tile([64, 512], f32, tag="m2_1", bufs=2, name="m2_1_init")
            nc.gpsimd.memset(M2[:], 0.0)
            for t in range(NSTEP):
                M1, M2 = _emit_step(nc, mybir, sb, ps, 1, t, M1, M2, cur1_1, consts)
            _emit_finals(nc, mybir, psum_out, M2, w3r, 1)

            # ---- output ----
            out_sb = sb.tile([128, 32], f32, name="out_sb")
            nc.vector.tensor_add(out_sb[:], psum_out[:, 0:32], b3bc[:])
            nc.sync.dma_start(out_d[:], out_sb[:])

    nc.compile()
    return nc


def get_nc():
    if "nc" not in _CACHE:
        _CACHE["nc"] = build_nc()
    return _CACHE["nc"]


def round_fp32r(x):
    """fp32 -> fp32r encoding (RNE to 11 stored mantissa bits, low 12 bits 0).

    Matches walrus fp32_to_fp32r: the PE's fast fp32 mode reads only the top
    20 bits; pre-rounding host-side keeps the DMA a pure byte copy."""
    u = np.ascontiguousarray(np.asarray(x, np.float32)).view(np.uint32)
    keep = u & np.uint32(0xFFFFF000)
    rb = (u & np.uint32(0x800)) != 0
    sticky = ((u & np.uint32(0x7FF)) != 0) | ((u & np.uint32(0x1000)) != 0)
    up = (rb & sticky).astype(np.uint32) * np.uint32(0x1000)
    return (keep + up).view(np.float32)


def host_inputs(x, W1, b1, W2, b2, W3, b3):
    """Per-core input dicts from full inputs."""
    x = np.asarray(x, np.float32)
    W1 = np.asarray(W1, np.float32)
    b1 = np.asarray(b1, np.float32)
    W2 = np.asarray(W2, np.float32)
    b2 = np.asarray(b2, np.float32)
    W3 = np.asarray(W3, np.float32)
    b3 = np.asarray(b3, np.float32)

    xT = round_fp32r(np.ascontiguousarray(x.T))  # [1700, 16384]
    w1T = W1.T  # [1700, 64]
    w1blk = np.zeros((128, NKCH * 128), np.float32)
    for c in range(NKCH):
        kr = KROWS[c]
        blk = w1T[64 * c : 64 * c + kr, :]
        w1blk[0:kr, 128 * c : 128 * c + 64] = -blk
        w1blk[kr : 2 * kr, 128 * c + 64 : 128 * c + 128] = -blk
    w1blk = round_fp32r(w1blk)
    b1q = round_fp32r((-np.concatenate([b1, b1]))[None, :].astype(np.float32))
    w2p = np.zeros((128, 64), np.float32)
    w2p[0:64, 0:32] = W2.T
    w2p[64:128, 32:64] = W2.T
    w2p = round_fp32r(w2p)
    b2qn = round_fp32r((-np.tile(b2, 2))[None, :].astype(np.float32))
    bI64 = round_fp32r((BETA * np.eye(64)).astype(np.float32))
    w3r = round_fp32r(np.tile(np.ascontiguousarray(-W3.T), (2, 1)).astype(np.float32))
    b3bc = np.ascontiguousarray(
        np.broadcast_to(np.tile(b3, 16)[None, :], (128, 32))
    ).astype(np.float32)

    in_maps = []
    for cc in range(N_CORES):
        in_maps.append(
            {
                "xt": np.ascontiguousarray(xT[:, cc * BC : (cc + 1) * BC]),
                "w1blk": w1blk,
                "b1q": b1q,
                "w2p": w2p,
                "b2qn": b2qn,
                "bI64": bI64,
                "w3r": w3r,
                "b3bc": b3bc,
                "ones": np.ones((1, 512), np.float32),
                "zrow": np.zeros((1, 128), np.float32),
            }
        )
    return in_maps


def assemble_output(per_core_outs):
    """per_core_outs: list of [128, 32] arrays -> [16384, 2]."""
    outs = []
    for o in per_core_outs:
        o = np.asarray(o, np.float32)
        outs.append(o.reshape(128, 16, 2).transpose(1, 0, 2).reshape(BC, 2))
    return np.concatenate(outs, axis=0)


def kernel(x, W1, b1, W2, b2, W3, b3, num_steps):
    assert int(num_steps) == NSTEP, f"kernel hardcodes num_steps={NSTEP}"
    from concourse.bass_utils import run_bass_kernel_spmd

    nc = get_nc()
    in_maps = host_inputs(x, W1, b1, W2, b2, W3, b3)
    res = run_bass_kernel_spmd(nc, in_maps, core_ids=list(range(N_CORES)))
    return assemble_output([res.results[c]["out"] for c in range(N_CORES)])


# revision 16
# speedup vs baseline: 2.0199x; 2.0199x over previous
"""DeepSNN Trainium2 kernel: 2-layer LIF SNN over 20 timesteps.

Strategy (pure data parallel over 8 cores, 2048 batch rows each):
- Host pre-transposes x -> xT [1700, 2048] per core so the contraction dim
  lands on SBUF partitions with fully contiguous DMAs (no on-device transpose).
- All matmuls run in fp32r (the PE's full-rate ~12-bit-mantissa fp32 mode);
  operands are pre-rounded host-side so DMAs stay pure byte copies, and the
  on-device state writes round on write via out-AP dtype.
- fp32r matmuls require dst PSUM base partition 0, so the big matmul packs two
  batch groups per column via a K=64 block-diagonal W1 (M=128), and the cur2
  matmuls write two column ranges of one [64, 512] PSUM bank.
- Membrane state is stored NEGATED (M = -mem) so one LIF update is exactly two
  fused scalar_tensor_tensor ops:
      t  = (M * beta) - cur          (POOL engine)
      M' = (M < -1) + t              (DVE engine)
- cur2 + beta*M2 - b2 accumulate fully in PSUM (beta*I64, block-diag W2, K=1
  bias matmuls), so the mem2 update is one DVE pass reading PSUM.
- Batch is split into 2 halves per core to pipeline DMA / big matmul /
  recurrent loop; half-1's big matmul interleaves into half-0's loop on PE.
"""

import sys

if "/opt/trn_rl_repo" not in sys.path:
    sys.path.insert(0, "/opt/trn_rl_repo")

import numpy as np

B_TOTAL = 16384
N_CORES = 8
BC = 2048  # batch per core
KDIM = 1700
NKCH = 27  # K chunks of 64 (26*64 + 36)
KROWS = [64] * 26 + [36]
HB = 1024  # batch per half
W = 512  # M1 free width per half
NSTEP = 20
BETA = 0.9

_CACHE = {}


CGROUPS = [(0, 4), (4, 4), (8, 4), (12, 4), (16, 4), (20, 4), (24, 2), (26, 1)]


def _emit_bigmm_group(nc, mybir, sb, xt_d, w1blk, psum_c1, h, g):
    """Load a group of K-chunks with two wide DMAs (outer dim = 64 partitions
    so descriptors spread across all SDMA engines), then one matmul per chunk.

    Group tile xk [128, 512*n]: col-block j holds chunk c0+j; rows 0-63 =
    batch group A rows (xT rows 64c..64c+kr), rows 64-127 = batch group B."""
    f32r = mybir.dt.float32r
    c0, nch = g
    kr = KROWS[c0 + nch - 1]  # only the last chunk may be short
    xk = sb.tile([128, 2048], f32r, tag="xk", bufs=3, name=f"xk_{h}_{c0}")
    for b in range(2):
        if nch > 1:
            src = xt_d[
                64 * c0 : 64 * (c0 + nch),
                HB * h + W * b : HB * h + W * b + W,
            ].rearrange("(c k) n -> k c n", c=nch)
            dst = xk[64 * b : 64 * b + 64, 0 : 512 * nch].rearrange(
                "k (c n) -> k c n", c=nch
            )
        else:
            src = xt_d[64 * c0 : 64 * c0 + kr, HB * h + W * b : HB * h + W * b + W]
            dst = xk[kr * b : kr * b + kr, 0:512]
        nc.sync.dma_start(dst, src)
    for j in range(nch):
        c = c0 + j
        krj = KROWS[c]
        rhs = xk[0 : 2 * krj, 512 * j : 512 * j + 512]
        lhs = w1blk[0 : 2 * krj, 128 * c : 128 * c + 128]
        nc.tensor.matmul(
            psum_c1[:, :], lhs, rhs, start=False, stop=False
        )


def _emit_step(nc, mybir, sb, ps, h, t, M1, M2, cur1, consts):
    f32 = mybir.dt.float32
    f32r = mybir.dt.float32r
    ALU = mybir.AluOpType
    w2p, b2qn, bI64, ones = consts

    u = sb.tile([128, W], f32, tag=f"u_{h}", bufs=2, name=f"u_{h}_{t}")
    nc.scalar.mul(u[:], M1[:], BETA)
    t1 = sb.tile([128, W], f32, tag=f"t1_{h}", bufs=2, name=f"t1_{h}_{t}")
    nc.gpsimd.tensor_add(t1[:], u[:], cur1[:])
    M1n = sb.tile([128, W], f32, tag=f"m1_{h}", bufs=2, name=f"m1_{h}_{t}")
    nc.vector.scalar_tensor_tensor(
        M1n[:].bitcast(f32r), M1[:], -1.0, t1[:], ALU.is_lt, ALU.add
    )
    pt2 = ps.tile([64, 512], f32, tag=f"t2_{h}", bufs=2, name=f"pt2_{h}_{t}")
    nc.tensor.matmul(
        pt2[:, :], bI64[:], M2[:].bitcast(f32r), start=True, stop=False
    )
    nc.tensor.matmul(
        pt2[:, 0:256], w2p[:], M1n[:, 0:256].bitcast(f32r),
        start=False, stop=False,
    )
    nc.tensor.matmul(
        pt2[:, 256:512], w2p[:], M1n[:, 256:512].bitcast(f32r),
        start=False, stop=False,
    )
    nc.tensor.matmul(
        pt2[:, :], b2qn[0:1, :], ones[0:1, :], start=False, stop=True
    )
    M2n = sb.tile([64, 512], f32, tag=f"m2_{h}", bufs=2, name=f"m2_{h}_{t}")
    nc.vector.scalar_tensor_tensor(
        M2n[:].bitcast(f32r), M2[:], -1.0, pt2[:, :], ALU.is_lt, ALU.add
    )
    return M1n, M2n


def _emit_finals(nc, mybir, psum_out, M2, w3r, h):
    f32r = mybir.dt.float32r
    for a in range(2):
        for b in range(2):
            for c in range(2):
                s = 8 * h + 4 * a + 2 * b + c
                first = h == 0 and a == 0 and b == 0 and c == 0
                last = h == 1 and a == 1 and b == 1 and c == 1
                off = 256 * b + 128 * c
                nc.tensor.matmul(
                    psum_out[:, 2 * s : 2 * s + 2],
                    M2[32 * a : 32 * a + 32, off : off + 128].bitcast(f32r),
                    w3r[32 * a : 32 * a + 32, :],
                    start=first,
                    stop=last,
                    tile_position=(32 * a, 0),
                )


def build_nc():
    import concourse.tile as tile
    from concourse import bacc, mybir

    f32 = mybir.dt.float32
    f32r = mybir.dt.float32r

    nc = bacc.Bacc("TRN2", target_bir_lowering=False, debug=False)
    xt_d = nc.dram_tensor("xt", [KDIM, BC], f32r, kind="ExternalInput")
    w1blk_d = nc.dram_tensor("w1blk", [128, NKCH * 128], f32r, kind="ExternalInput")
    b1q_d = nc.dram_tensor("b1q", [1, 128], f32r, kind="ExternalInput")
    w2p_d = nc.dram_tensor("w2p", [128, 64], f32r, kind="ExternalInput")
    b2qn_d = nc.dram_tensor("b2qn", [1, 64], f32r, kind="ExternalInput")
    bI64_d = nc.dram_tensor("bI64", [64, 64], f32r, kind="ExternalInput")
    w3r_d = nc.dram_tensor("w3r", [64, 2], f32r, kind="ExternalInput")
    b3bc_d = nc.dram_tensor("b3bc", [128, 32], f32, kind="ExternalInput")
    ones_d = nc.dram_tensor("ones", [1, 512], f32r, kind="ExternalInput")
    zrow_d = nc.dram_tensor("zrow", [1, 128], f32r, kind="ExternalInput")
    out_d = nc.dram_tensor("out", [128, 32], f32, kind="ExternalOutput")

    with tile.TileContext(nc) as tc:
        with (
            tc.tile_pool(name="sb", bufs=1) as sb,
            tc.tile_pool(name="ps", bufs=1, space="PSUM") as ps,
        ):
            # ---- constants ----
            w1blk = sb.tile([128, NKCH * 128], f32r, name="w1blk_sb")
            nc.sync.dma_start(w1blk[:], w1blk_d[:])
            b1q = sb.tile([1, 128], f32r, name="b1q_sb")
            nc.sync.dma_start(b1q[:], b1q_d[:])
            w2p = sb.tile([128, 64], f32r, name="w2p_sb")
            nc.sync.dma_start(w2p[:], w2p_d[:])
            b2qn = sb.tile([1, 64], f32r, name="b2qn_sb")
            nc.sync.dma_start(b2qn[:], b2qn_d[:])
            bI64 = sb.tile([64, 64], f32r, name="bI64_sb")
            nc.sync.dma_start(bI64[:], bI64_d[:])
            w3r = sb.tile([64, 2], f32r, name="w3r_sb")
            nc.sync.dma_start(w3r[:], w3r_d[:])
            b3bc = sb.tile([128, 32], f32, name="b3bc_sb")
            nc.sync.dma_start(b3bc[:], b3bc_d[:])
            ones = sb.tile([1, 512], f32r, name="ones_sb")
            nc.sync.dma_start(ones[:], ones_d[:])
            zrow = sb.tile([1, 128], f32r, name="zrow_sb")
            nc.sync.dma_start(zrow[:], zrow_d[:])
            consts = (w2p, b2qn, bI64, ones)

            psum_out = ps.tile([128, 512], f32, tag="out", bufs=1, name="psum_out")

            # ---- half 0: DMA + big matmul ----
            pc1_0 = ps.tile([128, 512], f32, tag="c1_0", bufs=1, name="pc1_0")
            nc.tensor.matmul(
                pc1_0[:, :], zrow[0:1, :], ones[0:1, :], start=True, stop=False
            )
            for g in CGROUPS:
                _emit_bigmm_group(nc, mybir, sb, xt_d, w1blk, pc1_0, 0, g)
            nc.tensor.matmul(
                pc1_0[:, :], b1q[0:1, :], ones[0:1, :], start=False, stop=True
            )
            cur1_0 = sb.tile([128, W], f32, name="cur1_0")
            nc.scalar.copy(cur1_0[:], pc1_0[:, :])

            # half-1 cur1 psum opened early so its chunks can slot in
            pc1_1 = ps.tile([128, 512], f32, tag="c1_1", bufs=1, name="pc1_1")
            nc.tensor.matmul(
                pc1_1[:, :], zrow[0:1, :], ones[0:1, :], start=True, stop=False
            )

            # ---- half 0 recurrent loop (half-1 big-mm interleaved on PE) ----
            M1 = sb.tile([128, W], f32, tag="m1_0", bufs=2, name="m1_0_init")
            nc.vector.memset(M1[:], 0.0)
            M2 = sb.tile([64, 512], f32, tag="m2_0", bufs=2, name="m2_0_init")
            nc.gpsimd.memset(M2[:], 0.0)
            for t in range(NSTEP):
                M1, M2 = _emit_step(nc, mybir, sb, ps, 0, t, M1, M2, cur1_0, consts)
                if t % 2 == 1 and (t - 1) // 2 < len(CGROUPS):
                    _emit_bigmm_group(
                        nc, mybir, sb, xt_d, w1blk, pc1_1, 1, CGROUPS[(t - 1) // 2]
                    )
            _emit_finals(nc, mybir, psum_out, M2, w3r, 0)

            # ---- half 1 ----
            nc.tensor.matmul(
                pc1_1[:, :], b1q[0:1, :], ones[0:1, :], start=False, stop=True
            )
            cur1_1 = sb.tile([128, W], f32, name="cur1_1")
            nc.scalar.copy(cur1_1[:], pc1_1[:, :])
            M1 = sb.tile([128, W], f32, tag="m1_1", bufs=2, name="m1_1_init")
            nc.vector.memset(M1[:], 0.0)
            M2 = sb.tile([64, 512], f32, tag="m2_1", bufs=2, name="m2_1_init")
            nc.gpsimd.memset(M2[:], 0.0)
            for t in range(NSTEP):
                M1, M2 = _emit_step(nc, mybir, sb, ps, 1, t, M1, M2, cur1_1, consts)
            _emit_finals(nc, mybir, psum_out, M2, w3r, 1)

            # ---- output ----
            out_sb = sb.tile([128, 32], f32, name="out_sb")
            nc.vector.tensor_add(out_sb[:], psum_out[:, 0:32], b3bc[:])
            nc.sync.dma_start(out_d[:], out_sb[:])

    nc.compile()
    return nc


def get_nc():
    if "nc" not in _CACHE:
        _CACHE["nc"] = build_nc()
    return _CACHE["nc"]


def round_fp32r(x):
    """fp32 -> fp32r encoding (RNE to 11 stored mantissa bits, low 12 bits 0).

    Matches walrus fp32_to_fp32r: the PE's fast fp32 mode reads only the top
    20 bits; pre-rounding host-side keeps the DMA a pure byte copy."""
    u = np.ascontiguousarray(np.asarray(x, np.float32)).view(np.uint32)
    keep = u & np.uint32(0xFFFFF000)
    rb = (u & np.uint32(0x800)) != 0
    sticky = ((u & np.uint32(0x7FF)) != 0) | ((u & np.uint32(0x1000)) != 0)
    up = (rb & sticky).astype(np.uint32) * np.uint32(0x1000)
    return (keep + up).view(np.float32)


def host_inputs(x, W1, b1, W2, b2, W3, b3):
    """Per-core input dicts from full inputs."""
    x = np.asarray(x, np.float32)
    W1 = np.asarray(W1, np.float32)
    b1 = np.asarray(b1, np.float32)
    W2 = np.asarray(W2, np.float32)
    b2 = np.asarray(b2, np.float32)
    W3 = np.asarray(W3, np.float32)
    b3 = np.asarray(b3, np.float32)

    xT = round_fp32r(np.ascontiguousarray(x.T))  # [1700, 16384]
    w1T = W1.T  # [1700, 64]
    w1blk = np.zeros((128, NKCH * 128), np.float32)
    for c in range(NKCH):
        kr = KROWS[c]
        blk = w1T[64 * c : 64 * c + kr, :]
        w1blk[0:kr, 128 * c : 128 * c + 64] = -blk
        w1blk[kr : 2 * kr, 128 * c + 64 : 128 * c + 128] = -blk
    w1blk = round_fp32r(w1blk)
    b1q = round_fp32r((-np.concatenate([b1, b1]))[None, :].astype(np.float32))
    w2p = np.zeros((128, 64), np.float32)
    w2p[0:64, 0:32] = W2.T
    w2p[64:128, 32:64] = W2.T
    w2p = round_fp32r(w2p)
    b2qn = round_fp32r((-np.tile(b2, 2))[None, :].astype(np.float32))
    bI64 = round_fp32r((BETA * np.eye(64)).astype(np.float32))
    w3r = round_fp32r(np.tile(np.ascontiguousarray(-W3.T), (2, 1)).astype(np.float32))
    b3bc = np.ascontiguousarray(
        np.broadcast_to(np.tile(b3, 16)[None, :], (128, 32))
    ).astype(np.float32)

    in_maps = []
    for cc in range(N_CORES):
        in_maps.append(
            {
                "xt": np.ascontiguousarray(xT[:, cc * BC : (cc + 1) * BC]),
                "w1blk": w1blk,
                "b1q": b1q,
                "w2p": w2p,
                "b2qn": b2qn,
                "bI64": bI64,
                "w3r": w3r,
                "b3bc": b3bc,
                "ones": np.ones((1, 512), np.float32),
                "zrow": np.zeros((1, 128), np.float32),
            }
        )
    return in_maps


def assemble_output(per_core_outs):
    """per_core_outs: list of [128, 32] arrays -> [16384, 2]."""
    outs = []
    for o in per_core_outs:
        o = np.asarray(o, np.float32)
        outs.append(o.reshape(128, 16, 2).transpose(1, 0, 2).reshape(BC, 2))
    return np.concatenate(outs, axis=0)


def kernel(x, W1, b1, W2, b2, W3, b3, num_steps):
    assert int(num_steps) == NSTEP, f"kernel hardcodes num_steps={NSTEP}"
    from concourse.bass_utils import run_bass_kernel_spmd

    nc = get_nc()
    in_maps = host_inputs(x, W1, b1, W2, b2, W3, b3)
    res = run_bass_kernel_spmd(nc, in_maps, core_ids=list(range(N_CORES)))
    return assemble_output([res.results[c]["out"] for c in range(N_CORES)])


# revision 17
# speedup vs baseline: 2.4470x; 1.2114x over previous
"""DeepSNN Trainium2 kernel: 2-layer LIF SNN over 20 timesteps.

Strategy (pure data parallel over 8 cores, 2048 batch rows each):
- Host pre-transposes x -> xT [1700, 2048] per core so the contraction dim
  lands on SBUF partitions with fully contiguous DMAs (no on-device transpose).
- All matmuls run in fp32r (the PE's full-rate ~12-bit-mantissa fp32 mode);
  operands are pre-rounded host-side so DMAs stay pure byte copies, and the
  on-device state writes round on write via out-AP dtype.
- fp32r matmuls require dst PSUM base partition 0, so the big matmul packs two
  batch groups per column via a K=64 block-diagonal W1 (M=128), and the cur2
  matmuls write two column ranges of one [64, 512] PSUM bank.
- Membrane state is stored NEGATED (M = -mem) so one LIF update is exactly two
  fused scalar_tensor_tensor ops:
      t  = (M * beta) - cur          (POOL engine)
      M' = (M < -1) + t              (DVE engine)
- cur2 + beta*M2 - b2 accumulate fully in PSUM (beta*I64, block-diag W2, K=1
  bias matmuls), so the mem2 update is one DVE pass reading PSUM.
- Batch is split into 2 halves per core to pipeline DMA / big matmul /
  recurrent loop; half-1's big matmul interleaves into half-0's loop on PE.
"""

import sys

if "/opt/trn_rl_repo" not in sys.path:
    sys.path.insert(0, "/opt/trn_rl_repo")

import numpy as np

B_TOTAL = 16384
N_CORES = 8
BC = 2048  # batch per core
KDIM = 1700
NKCH = 28  # K chunks of 64; chunk 26 zero-padded past row 1700, chunk 27 all pad
NMM = 27  # matmuls per half (chunk 27 is all padding, no matmul)
KPAD = 1792  # 28 * 64
HB = 1024  # batch per half
W = 512  # M1 free width per half
NSTEP = 20
BETA = 0.9

_CACHE = {}


NGROUPS = 7  # groups of 4 chunks, [128, 2048] prepacked DMA blocks


def _emit_bigmm_group(nc, mybir, sb, xt_d, w1blk, psum_c1, h, g):
    """One plain [128, 2048] DMA from the host-prepacked block layout (8 KiB
    contiguous per partition -> full descriptor efficiency), then one matmul
    per real chunk. Block (h, g) row p, col 512j+n holds
    xT[64*(4g+j) + (p%64), 1024h + 512*(p//64) + n] (zero-padded past K=1700).
    """
    f32r = mybir.dt.float32r
    xk = sb.tile([128, 2048], f32r, tag="xk", bufs=3, name=f"xk_{h}_{g}")
    nc.sync.dma_start(xk[:], xt_d[128 * (NGROUPS * h + g) : 128 * (NGROUPS * h + g) + 128, :])
    for j in range(4):
        c = 4 * g + j
        if c >= NMM:
            continue
        nc.tensor.matmul(
            psum_c1[:, :],
            w1blk[:, 128 * c : 128 * c + 128],
            xk[:, 512 * j : 512 * j + 512],
            start=False,
            stop=False,
        )


def _emit_step(nc, mybir, sb, ps, h, t, M1, M2, cur1, consts):
    f32 = mybir.dt.float32
    f32r = mybir.dt.float32r
    ALU = mybir.AluOpType
    w2p, b2qn, bI64, ones = consts

    t1 = sb.tile([128, W], f32, tag=f"t1_{h}", bufs=2, name=f"t1_{h}_{t}")
    nc.vector.scalar_tensor_tensor(
        t1[:], M1[:], BETA, cur1[:], ALU.mult, ALU.add
    )
    M1n = sb.tile([128, W], f32, tag=f"m1_{h}", bufs=2, name=f"m1_{h}_{t}")
    nc.vector.scalar_tensor_tensor(
        M1n[:].bitcast(f32r), M1[:], -1.0, t1[:], ALU.is_lt, ALU.add
    )
    pt2 = ps.tile([64, 512], f32, tag=f"t2_{h}", bufs=2, name=f"pt2_{h}_{t}")
    nc.tensor.matmul(
        pt2[:, :], bI64[:], M2[:].bitcast(f32r), start=True, stop=False
    )
    nc.tensor.matmul(
        pt2[:, 0:256], w2p[:], M1n[:, 0:256].bitcast(f32r),
        start=False, stop=False,
    )
    nc.tensor.matmul(
        pt2[:, 256:512], w2p[:], M1n[:, 256:512].bitcast(f32r),
        start=False, stop=True,
    )
    c2t = sb.tile([64, 512], f32, tag=f"c2t_{h}", bufs=2, name=f"c2t_{h}_{t}")
    nc.scalar.activation(
        c2t[:], pt2[:, :], mybir.ActivationFunctionType.Identity,
        bias=b2qn[:, 0:1], scale=1.0,
    )
    M2n = sb.tile([64, 512], f32, tag=f"m2_{h}", bufs=2, name=f"m2_{h}_{t}")
    nc.vector.scalar_tensor_tensor(
        M2n[:].bitcast(f32r), M2[:], -1.0, c2t[:], ALU.is_lt, ALU.add
    )
    return M1n, M2n


def _emit_finals(nc, mybir, psum_out, M2, w3r, h):
    f32r = mybir.dt.float32r
    for a in range(2):
        for b in range(2):
            for c in range(2):
                s = 8 * h + 4 * a + 2 * b + c
                first = h == 0 and a == 0 and b == 0 and c == 0
                last = h == 1 and a == 1 and b == 1 and c == 1
                off = 256 * b + 128 * c
                nc.tensor.matmul(
                    psum_out[:, 2 * s : 2 * s + 2],
                    M2[32 * a : 32 * a + 32, off : off + 128].bitcast(f32r),
                    w3r[32 * a : 32 * a + 32, :],
                    start=first,
                    stop=last,
                    tile_position=(32 * a, 0),
                )


def build_nc():
    import concourse.tile as tile
    from concourse import bacc, mybir

    f32 = mybir.dt.float32
    f32r = mybir.dt.float32r

    nc = bacc.Bacc("TRN2", target_bir_lowering=False, debug=False)
    xt_d = nc.dram_tensor("xt", [2 * NGROUPS * 128, 2048], f32r, kind="ExternalInput")
    w1blk_d = nc.dram_tensor("w1blk", [128, NKCH * 128], f32r, kind="ExternalInput")
    b1q_d = nc.dram_tensor("b1q", [1, 128], f32r, kind="ExternalInput")
    w2p_d = nc.dram_tensor("w2p", [128, 64], f32r, kind="ExternalInput")
    b2qn_d = nc.dram_tensor("b2qn", [64, 1], f32, kind="ExternalInput")
    bI64_d = nc.dram_tensor("bI64", [64, 64], f32r, kind="ExternalInput")
    w3r_d = nc.dram_tensor("w3r", [64, 2], f32r, kind="ExternalInput")
    b3bc_d = nc.dram_tensor("b3bc", [128, 32], f32, kind="ExternalInput")
    ones_d = nc.dram_tensor("ones", [1, 512], f32r, kind="ExternalInput")
    zrow_d = nc.dram_tensor("zrow", [1, 128], f32r, kind="ExternalInput")
    out_d = nc.dram_tensor("out", [128, 32], f32, kind="ExternalOutput")

    with tile.TileContext(nc) as tc:
        with (
            tc.tile_pool(name="sb", bufs=1) as sb,
            tc.tile_pool(name="ps", bufs=1, space="PSUM") as ps,
        ):
            # ---- constants ----
            w1blk = sb.tile([128, NKCH * 128], f32r, name="w1blk_sb")
            nc.sync.dma_start(w1blk[:], w1blk_d[:])
            b1q = sb.tile([1, 128], f32r, name="b1q_sb")
            nc.sync.dma_start(b1q[:], b1q_d[:])
            w2p = sb.tile([128, 64], f32r, name="w2p_sb")
            nc.sync.dma_start(w2p[:], w2p_d[:])
            b2qn = sb.tile([64, 1], f32, name="b2qn_sb")
            nc.sync.dma_start(b2qn[:], b2qn_d[:])
            bI64 = sb.tile([64, 64], f32r, name="bI64_sb")
            nc.sync.dma_start(bI64[:], bI64_d[:])
            w3r = sb.tile([64, 2], f32r, name="w3r_sb")
            nc.sync.dma_start(w3r[:], w3r_d[:])
            b3bc = sb.tile([128, 32], f32, name="b3bc_sb")
            nc.sync.dma_start(b3bc[:], b3bc_d[:])
            ones = sb.tile([1, 512], f32r, name="ones_sb")
            nc.sync.dma_start(ones[:], ones_d[:])
            zrow = sb.tile([1, 128], f32r, name="zrow_sb")
            nc.sync.dma_start(zrow[:], zrow_d[:])
            consts = (w2p, b2qn, bI64, ones)

            psum_out = ps.tile([128, 512], f32, tag="out", bufs=1, name="psum_out")

            # ---- half 0: DMA + big matmul ----
            pc1_0 = ps.tile([128, 512], f32, tag="c1_0", bufs=1, name="pc1_0")
            nc.tensor.matmul(
                pc1_0[:, :], zrow[0:1, :], ones[0:1, :], start=True, stop=False
            )
            for g in range(NGROUPS):
                _emit_bigmm_group(nc, mybir, sb, xt_d, w1blk, pc1_0, 0, g)
            nc.tensor.matmul(
                pc1_0[:, :], b1q[0:1, :], ones[0:1, :], start=False, stop=True
            )
            cur1_0 = sb.tile([128, W], f32, name="cur1_0")
            nc.scalar.copy(cur1_0[:], pc1_0[:, :])

            # half-1 cur1 psum opened early so its chunks can slot in
            pc1_1 = ps.tile([128, 512], f32, tag="c1_1", bufs=1, name="pc1_1")
            nc.tensor.matmul(
                pc1_1[:, :], zrow[0:1, :], ones[0:1, :], start=True, stop=False
            )

            # ---- half 0 recurrent loop (half-1 big-mm interleaved on PE) ----
            M1 = sb.tile([128, W], f32, tag="m1_0", bufs=2, name="m1_0_init")
            nc.vector.memset(M1[:], 0.0)
            M2 = sb.tile([64, 512], f32, tag="m2_0", bufs=2, name="m2_0_init")
            nc.gpsimd.memset(M2[:], 0.0)
            for t in range(NSTEP):
                M1, M2 = _emit_step(nc, mybir, sb, ps, 0, t, M1, M2, cur1_0, consts)
                if t % 2 == 1 and (t - 1) // 2 < NGROUPS:
                    _emit_bigmm_group(
                        nc, mybir, sb, xt_d, w1blk, pc1_1, 1, (t - 1) // 2
                    )
            _emit_finals(nc, mybir, psum_out, M2, w3r, 0)

            # ---- half 1 ----
            nc.tensor.matmul(
                pc1_1[:, :], b1q[0:1, :], ones[0:1, :], start=False, stop=True
            )
            cur1_1 = sb.tile([128, W], f32, name="cur1_1")
            nc.scalar.copy(cur1_1[:], pc1_1[:, :])
            M1 = sb.tile([128, W], f32, tag="m1_1", bufs=2, name="m1_1_init")
            nc.vector.memset(M1[:], 0.0)
            M2 = sb.tile([64, 512], f32, tag="m2_1", bufs=2, name="m2_1_init")
            nc.gpsimd.memset(M2[:], 0.0)
            for t in range(NSTEP):
                M1, M2 = _emit_step(nc, mybir, sb, ps, 1, t, M1, M2, cur1_1, consts)
            _emit_finals(nc, mybir, psum_out, M2, w3r, 1)

            # ---- output ----
            out_sb = sb.tile([128, 32], f32, name="out_sb")
            nc.vector.tensor_add(out_sb[:], psum_out[:, 0:32], b3bc[:])
            nc.sync.dma_start(out_d[:], out_sb[:])

    nc.compile()
    return nc


def get_nc():
    if "nc" not in _CACHE:
        _CACHE["nc"] = build_nc()
    return _CACHE["nc"]


def round_fp32r(x):
    """fp32 -> fp32r encoding (RNE to 11 stored mantissa bits, low 12 bits 0).

    Matches walrus fp32_to_fp32r: the PE's fast fp32 mode reads only the top
    20 bits; pre-rounding host-side keeps the DMA a pure byte copy."""
    u = np.ascontiguousarray(np.asarray(x, np.float32)).view(np.uint32)
    keep = u & np.uint32(0xFFFFF000)
    rb = (u & np.uint32(0x800)) != 0
    sticky = ((u & np.uint32(0x7FF)) != 0) | ((u & np.uint32(0x1000)) != 0)
    up = (rb & sticky).astype(np.uint32) * np.uint32(0x1000)
    return (keep + up).view(np.float32)


def host_inputs(x, W1, b1, W2, b2, W3, b3):
    """Per-core input dicts from full inputs."""
    x = np.asarray(x, np.float32)
    W1 = np.asarray(W1, np.float32)
    b1 = np.asarray(b1, np.float32)
    W2 = np.asarray(W2, np.float32)
    b2 = np.asarray(b2, np.float32)
    W3 = np.asarray(W3, np.float32)
    b3 = np.asarray(b3, np.float32)

    xTp = np.zeros((KPAD, B_TOTAL), np.float32)
    xTp[0:KDIM, :] = x.T
    xTp = round_fp32r(xTp)
    # prepacked per-core block layout: [core][h][g][p=(b*64+k)][512j+n]
    # = xT[64*(4g+j)+k, core*2048 + 1024h + 512b + n]
    xblk = xTp.reshape(NKCH, 64, N_CORES, 2, 2, 512)  # (c,k,core,h,b,n)
    xblk = xblk.reshape(NGROUPS, 4, 64, N_CORES, 2, 2, 512)  # (g,j,k,core,h,b,n)
    xblk = np.ascontiguousarray(
        xblk.transpose(3, 4, 0, 5, 2, 1, 6)  # (core,h,g,b,k,j,n)
    ).reshape(N_CORES, 2 * NGROUPS * 128, 2048)
    w1T = W1.T  # [1700, 64]
    w1blk = np.zeros((128, NKCH * 128), np.float32)
    for c in range(NMM):
        kr = min(64, KDIM - 64 * c)
        blk = w1T[64 * c : 64 * c + kr, :]
        w1blk[0:kr, 128 * c : 128 * c + 64] = -blk
        w1blk[64 : 64 + kr, 128 * c + 64 : 128 * c + 128] = -blk
    w1blk = round_fp32r(w1blk)
    b1q = round_fp32r((-np.concatenate([b1, b1]))[None, :].astype(np.float32))
    w2p = np.zeros((128, 64), np.float32)
    w2p[0:64, 0:32] = W2.T
    w2p[64:128, 32:64] = W2.T
    w2p = round_fp32r(w2p)
    b2qn = (-np.tile(b2, 2))[:, None].astype(np.float32)
    bI64 = round_fp32r((BETA * np.eye(64)).astype(np.float32))
    w3r = round_fp32r(np.tile(np.ascontiguousarray(-W3.T), (2, 1)).astype(np.float32))
    b3bc = np.ascontiguousarray(
        np.broadcast_to(np.tile(b3, 16)[None, :], (128, 32))
    ).astype(np.float32)

    in_maps = []
    for cc in range(N_CORES):
        in_maps.append(
            {
                "xt": np.ascontiguousarray(xblk[cc]),
                "w1blk": w1blk,
                "b1q": b1q,
                "w2p": w2p,
                "b2qn": b2qn,
                "bI64": bI64,
                "w3r": w3r,
                "b3bc": b3bc,
                "ones": np.ones((1, 512), np.float32),
                "zrow": np.zeros((1, 128), np.float32),
            }
        )
    return in_maps


def assemble_output(per_core_outs):
    """per_core_outs: list of [128, 32] arrays -> [16384, 2]."""
    outs = []
    for o in per_core_outs:
        o = np.asarray(o, np.float32)
        outs.append(o.reshape(128, 16, 2).transpose(1, 0, 2).reshape(BC, 2))
    return np.concatenate(outs, axis=0)


def kernel(x, W1, b1, W2, b2, W3, b3, num_steps):
    assert int(num_steps) == NSTEP, f"kernel hardcodes num_steps={NSTEP}"
    from concourse.bass_utils import run_bass_kernel_spmd

    nc = get_nc()
    in_maps = host_inputs(x, W1, b1, W2, b2, W3, b3)
    res = run_bass_kernel_spmd(nc, in_maps, core_ids=list(range(N_CORES)))
    return assemble_output([res.results[c]["out"] for c in range(N_CORES)])


# revision 22
# speedup vs baseline: 2.6710x; 1.0916x over previous
"""DeepSNN Trainium2 kernel: 2-layer LIF SNN over 20 timesteps.

Strategy (pure data parallel over 8 cores, 2048 batch rows each):
- Host pre-transposes x -> xT [1700, 2048] per core so the contraction dim
  lands on SBUF partitions with fully contiguous DMAs (no on-device transpose).
- All matmuls run in fp32r (the PE's full-rate ~12-bit-mantissa fp32 mode);
  operands are pre-rounded host-side so DMAs stay pure byte copies, and the
  on-device state writes round on write via out-AP dtype.
- fp32r matmuls require dst PSUM base partition 0, so the big matmul packs two
  batch groups per column via a K=64 block-diagonal W1 (M=128), and the cur2
  matmuls write two column ranges of one [64, 512] PSUM bank.
- Membrane state is stored NEGATED (M = -mem) so one LIF update is exactly two
  fused scalar_tensor_tensor ops:
      t  = (M * beta) - cur          (POOL engine)
      M' = (M < -1) + t              (DVE engine)
- cur2 + beta*M2 - b2 accumulate fully in PSUM (beta*I64, block-diag W2, K=1
  bias matmuls), so the mem2 update is one DVE pass reading PSUM.
- Batch is split into 2 halves per core to pipeline DMA / big matmul /
  recurrent loop; half-1's big matmul interleaves into half-0's loop on PE.
"""

import sys

if "/opt/trn_rl_repo" not in sys.path:
    sys.path.insert(0, "/opt/trn_rl_repo")

import numpy as np

B_TOTAL = 16384
N_CORES = 8
BC = 2048  # batch per core
KDIM = 1700
NKCH = 28  # K chunks of 64; chunk 26 zero-padded past row 1700, chunk 27 all pad
NMM = 27  # matmuls per half (chunk 27 is all padding, no matmul)
KPAD = 1792  # 28 * 64
HB = 1024  # batch per half
W = 512  # M1 free width per half
NSTEP = 20
BETA = 0.9

_CACHE = {}


NGROUPS = 7  # groups of 4 chunks, [128, 2048] prepacked DMA blocks


def _emit_bigmm_group(nc, mybir, sb, xt_d, w1blk, psum_c1, h, g):
    """One plain [128, 2048] DMA from the host-prepacked block layout (8 KiB
    contiguous per partition -> full descriptor efficiency), then one matmul
    per real chunk. Block (h, g) row p, col 512j+n holds
    xT[64*(4g+j) + (p%64), 1024h + 512*(p//64) + n] (zero-padded past K=1700).
    """
    f32r = mybir.dt.float32r
    xk = sb.tile([128, 2048], f32r, tag="xk", bufs=3, name=f"xk_{h}_{g}")
    nc.sync.dma_start(xk[:], xt_d[128 * (NGROUPS * h + g) : 128 * (NGROUPS * h + g) + 128, :])
    for j in range(4):
        c = 4 * g + j
        if c >= NMM:
            continue
        nc.tensor.matmul(
            psum_c1[:, :],
            w1blk[:, 128 * c : 128 * c + 128],
            xk[:, 512 * j : 512 * j + 512],
            start=False,
            stop=False,
        )


def _emit_step(nc, mybir, sb, ps, h, t, M1, M2, cur1, consts):
    f32 = mybir.dt.float32
    f32r = mybir.dt.float32r
    ALU = mybir.AluOpType
    w2p, bI64, ones = consts

    t1 = sb.tile([128, W], f32, tag=f"t1_{h}", bufs=2, name=f"t1_{h}_{t}")
    nc.vector.scalar_tensor_tensor(
        t1[:], M1[:], BETA, cur1[:], ALU.mult, ALU.add
    )
    M1n = sb.tile([128, W], f32, tag=f"m1_{h}", bufs=2, name=f"m1_{h}_{t}")
    nc.vector.scalar_tensor_tensor(
        M1n[:].bitcast(f32r), M1[:], -1.0, t1[:], ALU.is_lt, ALU.add
    )
    pt2 = ps.tile([64, 512], f32, tag=f"t2_{h}", bufs=2, name=f"pt2_{h}_{t}")
    nc.tensor.matmul(
        pt2[:, :], bI64[:], M2[0:65, :].bitcast(f32r), start=True, stop=False
    )
    nc.tensor.matmul(
        pt2[:, 0:256], w2p[:], M1n[:, 0:256].bitcast(f32r),
        start=False, stop=False,
    )
    nc.tensor.matmul(
        pt2[:, 256:512], w2p[:], M1n[:, 256:512].bitcast(f32r),
        start=False, stop=True,
    )
    M2n = sb.tile([65, 512], f32, tag=f"m2_{h}", bufs=2, name=f"m2_{h}_{t}")
    nc.gpsimd.memset(M2n[64:65, :], 1.0)
    nc.vector.scalar_tensor_tensor(
        M2n[0:64, :].bitcast(f32r), M2[0:64, :], -1.0, pt2[:, :],
        ALU.is_lt, ALU.add
    )
    return M1n, M2n


def _emit_finals(nc, mybir, psum_out, M2, w3r, h):
    f32r = mybir.dt.float32r
    for a in range(2):
        for b in range(2):
            for c in range(2):
                s = 8 * h + 4 * a + 2 * b + c
                first = h == 0 and a == 0 and b == 0 and c == 0
                last = h == 1 and a == 1 and b == 1 and c == 1
                off = 256 * b + 128 * c
                nc.tensor.matmul(
                    psum_out[:, 2 * s : 2 * s + 2],
                    M2[32 * a : 32 * a + 32, off : off + 128].bitcast(f32r),
                    w3r[32 * a : 32 * a + 32, :],
                    start=first,
                    stop=last,
                    tile_position=(32 * a, 0),
                )


def build_nc():
    import concourse.tile as tile
    from concourse import bacc, mybir

    f32 = mybir.dt.float32
    f32r = mybir.dt.float32r

    nc = bacc.Bacc("TRN2", target_bir_lowering=False, debug=False)
    xt_d = nc.dram_tensor("xt", [2 * NGROUPS * 128, 2048], f32r, kind="ExternalInput")
    w1blk_d = nc.dram_tensor("w1blk", [128, NKCH * 128], f32r, kind="ExternalInput")
    b1q_d = nc.dram_tensor("b1q", [1, 128], f32r, kind="ExternalInput")
    w2p_d = nc.dram_tensor("w2p", [128, 64], f32r, kind="ExternalInput")
    bI64_d = nc.dram_tensor("bI64", [65, 64], f32r, kind="ExternalInput")
    w3r_d = nc.dram_tensor("w3r", [64, 2], f32r, kind="ExternalInput")
    b3bc_d = nc.dram_tensor("b3bc", [128, 32], f32, kind="ExternalInput")
    ones_d = nc.dram_tensor("ones", [1, 512], f32r, kind="ExternalInput")
    zrow_d = nc.dram_tensor("zrow", [1, 128], f32r, kind="ExternalInput")
    warm_d = nc.dram_tensor("warm", [128, 128], f32r, kind="ExternalInput")
    out_d = nc.dram_tensor("out", [128, 32], f32, kind="ExternalOutput")

    with tile.TileContext(nc) as tc:
        with (
            tc.tile_pool(name="sb", bufs=1) as sb,
            tc.tile_pool(name="ps", bufs=1, space="PSUM") as ps,
        ):
            # ---- constants ----
            w1blk = sb.tile([128, NKCH * 128], f32r, name="w1blk_sb")
            nc.scalar.dma_start(w1blk[:], w1blk_d[:])
            b1q = sb.tile([1, 128], f32r, name="b1q_sb")
            nc.scalar.dma_start(b1q[:], b1q_d[:])
            w2p = sb.tile([128, 64], f32r, name="w2p_sb")
            nc.scalar.dma_start(w2p[:], w2p_d[:])
            bI64 = sb.tile([65, 64], f32r, name="bI64_sb")
            nc.scalar.dma_start(bI64[:], bI64_d[:])
            w3r = sb.tile([64, 2], f32r, name="w3r_sb")
            nc.scalar.dma_start(w3r[:], w3r_d[:])
            b3bc = sb.tile([128, 32], f32, name="b3bc_sb")
            nc.scalar.dma_start(b3bc[:], b3bc_d[:])
            ones = sb.tile([1, 512], f32r, name="ones_sb")
            nc.scalar.dma_start(ones[:], ones_d[:])
            zrow = sb.tile([1, 128], f32r, name="zrow_sb")
            nc.scalar.dma_start(zrow[:], zrow_d[:])
            consts = (w2p, bI64, ones)

            psum_out = ps.tile([128, 512], f32, tag="out", bufs=1, name="psum_out")

            # PE warmup: keep the HAM busy while the first DMAs stream in
            warm = sb.tile([128, 128], f32r, name="warm_sb")
            nc.sync.dma_start(warm[:], warm_d[:])
            psum_w = ps.tile([128, 512], f32, tag="warm", bufs=1, name="psum_w")
            for i in range(16):
                nc.tensor.matmul(
                    psum_w[:, 0:128], warm[:], warm[:],
                    start=(i == 0), stop=(i == 15),
                )

            # ---- half 0: DMA + big matmul ----
            pc1_0 = ps.tile([128, 512], f32, tag="c1_0", bufs=1, name="pc1_0")
            nc.tensor.matmul(
                pc1_0[:, :], zrow[0:1, :], ones[0:1, :], start=True, stop=False
            )
            for g in range(NGROUPS):
                _emit_bigmm_group(nc, mybir, sb, xt_d, w1blk, pc1_0, 0, g)
            nc.tensor.matmul(
                pc1_0[:, :], b1q[0:1, :], ones[0:1, :], start=False, stop=True
            )
            cur1_0 = sb.tile([128, W], f32, name="cur1_0")
            nc.scalar.copy(cur1_0[:], pc1_0[:, :])

            # half-1 cur1 psum opened early so its chunks can slot in
            pc1_1 = ps.tile([128, 512], f32, tag="c1_1", bufs=1, name="pc1_1")
            nc.tensor.matmul(
                pc1_1[:, :], zrow[0:1, :], ones[0:1, :], start=True, stop=False
            )

            # ---- half 0 recurrent loop (half-1 big-mm interleaved on PE) ----
            M1 = sb.tile([128, W], f32, tag="m1_0", bufs=2, name="m1_0_init")
            nc.vector.memset(M1[:], 0.0)
            M2 = sb.tile([65, 512], f32, tag="m2_0", bufs=2, name="m2_0_init")
            nc.gpsimd.memset(M2[0:64, :], 0.0)
            nc.vector.memset(M2[64:65, :], 1.0)
            for t in range(NSTEP):
                M1, M2 = _emit_step(nc, mybir, sb, ps, 0, t, M1, M2, cur1_0, consts)
                if t % 2 == 1 and (t - 1) // 2 < NGROUPS:
                    _emit_bigmm_group(
                        nc, mybir, sb, xt_d, w1blk, pc1_1, 1, (t - 1) // 2
                    )
            _emit_finals(nc, mybir, psum_out, M2, w3r, 0)

            # ---- half 1 ----
            nc.tensor.matmul(
                pc1_1[:, :], b1q[0:1, :], ones[0:1, :], start=False, stop=True
            )
            cur1_1 = sb.tile([128, W], f32, name="cur1_1")
            nc.scalar.copy(cur1_1[:], pc1_1[:, :])
            M1 = sb.tile([128, W], f32, tag="m1_1", bufs=2, name="m1_1_init")
            nc.vector.memset(M1[:], 0.0)
            M2 = sb.tile([65, 512], f32, tag="m2_1", bufs=2, name="m2_1_init")
            nc.gpsimd.memset(M2[0:64, :], 0.0)
            nc.vector.memset(M2[64:65, :], 1.0)
            for t in range(NSTEP):
                M1, M2 = _emit_step(nc, mybir, sb, ps, 1, t, M1, M2, cur1_1, consts)
            _emit_finals(nc, mybir, psum_out, M2, w3r, 1)

            # ---- output ----
            out_sb = sb.tile([128, 32], f32, name="out_sb")
            nc.vector.tensor_add(out_sb[:], psum_out[:, 0:32], b3bc[:])
            nc.sync.dma_start(out_d[:], out_sb[:])

    nc.compile()
    return nc


def get_nc():
    if "nc" not in _CACHE:
        _CACHE["nc"] = build_nc()
    return _CACHE["nc"]


def round_fp32r(x):
    """fp32 -> fp32r encoding (RNE to 11 stored mantissa bits, low 12 bits 0).

    Matches walrus fp32_to_fp32r: the PE's fast fp32 mode reads only the top
    20 bits; pre-rounding host-side keeps the DMA a pure byte copy."""
    u = np.ascontiguousarray(np.asarray(x, np.float32)).view(np.uint32)
    keep = u & np.uint32(0xFFFFF000)
    rb = (u & np.uint32(0x800)) != 0
    sticky = ((u & np.uint32(0x7FF)) != 0) | ((u & np.uint32(0x1000)) != 0)
    up = (rb & sticky).astype(np.uint32) * np.uint32(0x1000)
    return (keep + up).view(np.float32)


def host_inputs(x, W1, b1, W2, b2, W3, b3):
    """Per-core input dicts from full inputs."""
    x = np.asarray(x, np.float32)
    W1 = np.asarray(W1, np.float32)
    b1 = np.asarray(b1, np.float32)
    W2 = np.asarray(W2, np.float32)
    b2 = np.asarray(b2, np.float32)
    W3 = np.asarray(W3, np.float32)
    b3 = np.asarray(b3, np.float32)

    xTp = np.zeros((KPAD, B_TOTAL), np.float32)
    xTp[0:KDIM, :] = x.T
    xTp = round_fp32r(xTp)
    # prepacked per-core block layout: [core][h][g][p=(b*64+k)][512j+n]
    # = xT[64*(4g+j)+k, core*2048 + 1024h + 512b + n]
    xblk = xTp.reshape(NKCH, 64, N_CORES, 2, 2, 512)  # (c,k,core,h,b,n)
    xblk = xblk.reshape(NGROUPS, 4, 64, N_CORES, 2, 2, 512)  # (g,j,k,core,h,b,n)
    xblk = np.ascontiguousarray(
        xblk.transpose(3, 4, 0, 5, 2, 1, 6)  # (core,h,g,b,k,j,n)
    ).reshape(N_CORES, 2 * NGROUPS * 128, 2048)
    w1T = W1.T  # [1700, 64]
    w1blk = np.zeros((128, NKCH * 128), np.float32)
    for c in range(NMM):
        kr = min(64, KDIM - 64 * c)
        blk = w1T[64 * c : 64 * c + kr, :]
        w1blk[0:kr, 128 * c : 128 * c + 64] = -blk
        w1blk[64 : 64 + kr, 128 * c + 64 : 128 * c + 128] = -blk
    w1blk = round_fp32r(w1blk)
    b1q = round_fp32r((-np.concatenate([b1, b1]))[None, :].astype(np.float32))
    w2p = np.zeros((128, 64), np.float32)
    w2p[0:64, 0:32] = W2.T
    w2p[64:128, 32:64] = W2.T
    w2p = round_fp32r(w2p)
    bI64 = round_fp32r(np.concatenate(
        [BETA * np.eye(64, dtype=np.float32), (-np.tile(b2, 2))[None, :]], axis=0
    ).astype(np.float32))
    w3r = round_fp32r(np.tile(np.ascontiguousarray(-W3.T), (2, 1)).astype(np.float32))
    b3bc = np.ascontiguousarray(
        np.broadcast_to(np.tile(b3, 16)[None, :], (128, 32))
    ).astype(np.float32)

    in_maps = []
    for cc in range(N_CORES):
        in_maps.append(
            {
                "xt": np.ascontiguousarray(xblk[cc]),
                "w1blk": w1blk,
                "b1q": b1q,
                "w2p": w2p,
                "bI64": bI64,
                "w3r": w3r,
                "b3bc": b3bc,
                "ones": np.ones((1, 512), np.float32),
                "zrow": np.zeros((1, 128), np.float32),
                "warm": np.zeros((128, 128), np.float32),
            }
        )
    return in_maps


def assemble_output(per_core_outs):
    """per_core_outs: list of [128, 32] arrays -> [16384, 2]."""
    outs = []
    for o in per_core_outs:
        o = np.asarray(o, np.float32)
        outs.append(o.reshape(128, 16, 2).transpose(1, 0, 2).reshape(BC, 2))
    return np.concatenate(outs, axis=0)


def kernel(x, W1, b1, W2, b2, W3, b3, num_steps):
    assert int(num_steps) == NSTEP, f"kernel hardcodes num_steps={NSTEP}"
    from concourse.bass_utils import run_bass_kernel_spmd

    nc = get_nc()
    in_maps = host_inputs(x, W1, b1, W2, b2, W3, b3)
    res = run_bass_kernel_spmd(nc, in_maps, core_ids=list(range(N_CORES)))
    return assemble_output([res.results[c]["out"] for c in range(N_CORES)])


# revision 23
# speedup vs baseline: 3.5165x; 1.3165x over previous
"""DeepSNN Trainium2 kernel: 2-layer LIF SNN over 20 timesteps.

Strategy (pure data parallel over 8 cores, 2048 batch rows each):
- Host pre-transposes x -> xT [1700, 2048] per core so the contraction dim
  lands on SBUF partitions with fully contiguous DMAs (no on-device transpose).
- All matmuls run in fp32r (the PE's full-rate ~12-bit-mantissa fp32 mode);
  operands are pre-rounded host-side so DMAs stay pure byte copies, and the
  on-device state writes round on write via out-AP dtype.
- fp32r matmuls require dst PSUM base partition 0, so the big matmul packs two
  batch groups per column via a K=64 block-diagonal W1 (M=128), and the cur2
  matmuls write two column ranges of one [64, 512] PSUM bank.
- Membrane state is stored NEGATED (M = -mem) so one LIF update is exactly two
  fused scalar_tensor_tensor ops:
      t  = (M * beta) - cur          (POOL engine)
      M' = (M < -1) + t              (DVE engine)
- cur2 + beta*M2 - b2 accumulate fully in PSUM (beta*I64, block-diag W2, K=1
  bias matmuls), so the mem2 update is one DVE pass reading PSUM.
- Batch is split into 2 halves per core to pipeline DMA / big matmul /
  recurrent loop; half-1's big matmul interleaves into half-0's loop on PE.
"""

import sys

if "/opt/trn_rl_repo" not in sys.path:
    sys.path.insert(0, "/opt/trn_rl_repo")

import numpy as np

B_TOTAL = 16384
N_CORES = 8
BC = 2048  # batch per core
KDIM = 1700
NKCH = 28  # K chunks of 64; chunk 26 zero-padded past row 1700, chunk 27 all pad
NMM = 27  # matmuls per half (chunk 27 is all padding, no matmul)
KPAD = 1792  # 28 * 64
HB = 1024  # batch per half
W = 512  # M1 free width per half
NSTEP = 20
BETA = 0.9

_CACHE = {}


def _register_lif_op():
    """Fused LIF update as one custom DVE op:
        out = (in0 * beta + in1) + (in0 < thresh)
    (state is negated, so this is the whole membrane update in one pass)."""
    import numpy as np
    from concourse.dve_spec import Spec, Src0, Src1, C0, C1
    from concourse import dve_ops as D

    if "LIF_FUSED_SNN" in D._SUB_OPCODE_FOR_NAME:
        return D._BY_NAME_LIF
    op = D.DveOp(
        "LIF_FUSED_SNN",
        Spec(
            body=(Src0 * C1 + Src1) + (Src0 < C0),
            reference=lambda in0, in1, s0, s1, imm2: (
                (in0 * s1 + in1) + (in0 < s0)
            ).astype(np.float32),
        ),
        subdim=False,
        uops_sha={"v3": "b45eee788a1e54cc", "v4": "8b27e300884d1856"},
    )
    D.OPS.append(op)
    D._SUB_OPCODE_FOR_NAME[op.name] = max(D._SUB_OPCODE_FOR_NAME.values()) + 1
    D.CUSTOM_DVE_SPECS[op.name] = op.spec
    D._BY_NAME_LIF = op
    return op


NGROUPS = 7  # groups of 4 chunks, [128, 2048] prepacked DMA blocks


def _emit_bigmm_group(nc, mybir, sb, xt_d, w1blk, psum_c1, h, g):
    """One plain [128, 2048] DMA from the host-prepacked block layout (8 KiB
    contiguous per partition -> full descriptor efficiency), then one matmul
    per real chunk. Block (h, g) row p, col 512j+n holds
    xT[64*(4g+j) + (p%64), 1024h + 512*(p//64) + n] (zero-padded past K=1700).
    """
    f32r = mybir.dt.float32r
    xk = sb.tile([128, 2048], f32r, tag="xk", bufs=4, name=f"xk_{h}_{g}")
    nc.sync.dma_start(xk[:], xt_d[128 * (NGROUPS * h + g) : 128 * (NGROUPS * h + g) + 128, :])
    for j in range(4):
        c = 4 * g + j
        if c >= NMM:
            continue
        nc.tensor.matmul(
            psum_c1[:, :],
            w1blk[:, 128 * c : 128 * c + 128],
            xk[:, 512 * j : 512 * j + 512],
            start=False,
            stop=False,
        )


def _emit_step(nc, mybir, sb, ps, h, t, M1, M2, cur1, consts, lif):
    f32 = mybir.dt.float32
    f32r = mybir.dt.float32r
    w2p, b2qn, ones = consts

    M1n = sb.tile([128, W], f32, tag=f"m1_{h}", bufs=2, name=f"m1_{h}_{t}")
    nc.vector._custom_dve(
        lif, out=M1n[:].bitcast(f32r), in0=M1[:], in1=cur1[:],
        s0=-1.0, s1=BETA,
    )
    pt2 = ps.tile([64, 512], f32, tag=f"t2_{h}", bufs=2, name=f"pt2_{h}_{t}")
    nc.tensor.matmul(
        pt2[:, :], b2qn[0:1, :], ones[0:1, :], start=True, stop=False
    )
    nc.tensor.matmul(
        pt2[:, 0:256], w2p[:], M1n[:, 0:256].bitcast(f32r),
        start=False, stop=False,
    )
    nc.tensor.matmul(
        pt2[:, 256:512], w2p[:], M1n[:, 256:512].bitcast(f32r),
        start=False, stop=True,
    )
    M2n = sb.tile([64, 512], f32, tag=f"m2_{h}", bufs=2, name=f"m2_{h}_{t}")
    nc.vector._custom_dve(
        lif, out=M2n[:].bitcast(f32r), in0=M2[:], in1=pt2[:, :],
        s0=-1.0, s1=BETA,
    )
    return M1n, M2n


def _emit_finals(nc, mybir, psum_out, M2, w3r, h):
    f32r = mybir.dt.float32r
    for a in range(2):
        for b in range(2):
            for c in range(2):
                s = 8 * h + 4 * a + 2 * b + c
                first = h == 0 and a == 0 and b == 0 and c == 0
                last = h == 1 and a == 1 and b == 1 and c == 1
                off = 256 * b + 128 * c
                nc.tensor.matmul(
                    psum_out[:, 2 * s : 2 * s + 2],
                    M2[32 * a : 32 * a + 32, off : off + 128].bitcast(f32r),
                    w3r[32 * a : 32 * a + 32, :],
                    start=first,
                    stop=last,
                    tile_position=(32 * a, 0),
                )


def build_nc():
    import concourse.tile as tile
    from concourse import bacc, mybir

    lif = _register_lif_op()

    f32 = mybir.dt.float32
    f32r = mybir.dt.float32r

    nc = bacc.Bacc("TRN2", target_bir_lowering=False, debug=False)
    xt_d = nc.dram_tensor("xt", [2 * NGROUPS * 128, 2048], f32r, kind="ExternalInput")
    w1blk_d = nc.dram_tensor("w1blk", [128, NKCH * 128], f32r, kind="ExternalInput")
    b1q_d = nc.dram_tensor("b1q", [1, 128], f32r, kind="ExternalInput")
    w2p_d = nc.dram_tensor("w2p", [128, 64], f32r, kind="ExternalInput")
    b2qn_d = nc.dram_tensor("b2qn", [1, 64], f32r, kind="ExternalInput")
    w3r_d = nc.dram_tensor("w3r", [64, 2], f32r, kind="ExternalInput")
    b3bc_d = nc.dram_tensor("b3bc", [128, 32], f32, kind="ExternalInput")
    ones_d = nc.dram_tensor("ones", [1, 512], f32r, kind="ExternalInput")
    zrow_d = nc.dram_tensor("zrow", [1, 128], f32r, kind="ExternalInput")
    warm_d = nc.dram_tensor("warm", [128, 128], f32r, kind="ExternalInput")
    out_d = nc.dram_tensor("out", [128, 32], f32, kind="ExternalOutput")

    with tile.TileContext(nc) as tc:
        with (
            tc.tile_pool(name="sb", bufs=1) as sb,
            tc.tile_pool(name="ps", bufs=1, space="PSUM") as ps,
        ):
            # ---- constants ----
            w1blk = sb.tile([128, NKCH * 128], f32r, name="w1blk_sb")
            nc.sync.dma_start(w1blk[:], w1blk_d[:])
            b1q = sb.tile([1, 128], f32r, name="b1q_sb")
            nc.scalar.dma_start(b1q[:], b1q_d[:])
            w2p = sb.tile([128, 64], f32r, name="w2p_sb")
            nc.scalar.dma_start(w2p[:], w2p_d[:])
            b2qn = sb.tile([1, 64], f32r, name="b2qn_sb")
            nc.scalar.dma_start(b2qn[:], b2qn_d[:])
            w3r = sb.tile([64, 2], f32r, name="w3r_sb")
            nc.scalar.dma_start(w3r[:], w3r_d[:])
            b3bc = sb.tile([128, 32], f32, name="b3bc_sb")
            nc.scalar.dma_start(b3bc[:], b3bc_d[:])
            ones = sb.tile([1, 512], f32r, name="ones_sb")
            nc.scalar.dma_start(ones[:], ones_d[:])
            zrow = sb.tile([1, 128], f32r, name="zrow_sb")
            nc.scalar.dma_start(zrow[:], zrow_d[:])
            consts = (w2p, b2qn, ones)

            psum_out = ps.tile([128, 512], f32, tag="out", bufs=1, name="psum_out")

            # PE warmup: keep the HAM busy while the first DMAs stream in
            warm = sb.tile([128, 128], f32r, name="warm_sb")
            nc.sync.dma_start(warm[:], warm_d[:])
            psum_w = ps.tile([128, 512], f32, tag="warm", bufs=1, name="psum_w")
            for i in range(16):
                nc.tensor.matmul(
                    psum_w[:, 0:128], warm[:], warm[:],
                    start=(i == 0), stop=(i == 15),
                )

            # ---- half 0: DMA + big matmul ----
            pc1_0 = ps.tile([128, 512], f32, tag="c1_0", bufs=1, name="pc1_0")
            nc.tensor.matmul(
                pc1_0[:, :], zrow[0:1, :], ones[0:1, :], start=True, stop=False
            )
            for g in range(NGROUPS):
                _emit_bigmm_group(nc, mybir, sb, xt_d, w1blk, pc1_0, 0, g)
            nc.tensor.matmul(
                pc1_0[:, :], b1q[0:1, :], ones[0:1, :], start=False, stop=True
            )
            cur1_0 = sb.tile([128, W], f32, name="cur1_0")
            nc.scalar.copy(cur1_0[:], pc1_0[:, :])

            # half-1 cur1 psum opened early so its chunks can slot in
            pc1_1 = ps.tile([128, 512], f32, tag="c1_1", bufs=1, name="pc1_1")
            nc.tensor.matmul(
                pc1_1[:, :], zrow[0:1, :], ones[0:1, :], start=True, stop=False
            )

            # ---- half 0 recurrent loop (half-1 big-mm interleaved on PE) ----
            M1 = sb.tile([128, W], f32, tag="m1_0", bufs=2, name="m1_0_init")
            nc.vector.memset(M1[:], 0.0)
            M2 = sb.tile([64, 512], f32, tag="m2_0", bufs=2, name="m2_0_init")
            nc.gpsimd.memset(M2[:], 0.0)
            for t in range(NSTEP):
                M1, M2 = _emit_step(nc, mybir, sb, ps, 0, t, M1, M2, cur1_0, consts, lif)
                if t % 2 == 1 and (t - 1) // 2 < NGROUPS:
                    _emit_bigmm_group(
                        nc, mybir, sb, xt_d, w1blk, pc1_1, 1, (t - 1) // 2
                    )
            _emit_finals(nc, mybir, psum_out, M2, w3r, 0)

            # ---- half 1 ----
            nc.tensor.matmul(
                pc1_1[:, :], b1q[0:1, :], ones[0:1, :], start=False, stop=True
            )
            cur1_1 = sb.tile([128, W], f32, name="cur1_1")
            nc.scalar.copy(cur1_1[:], pc1_1[:, :])
            M1 = sb.tile([128, W], f32, tag="m1_1", bufs=2, name="m1_1_init")
            nc.vector.memset(M1[:], 0.0)
            M2 = sb.tile([64, 512], f32, tag="m2_1", bufs=2, name="m2_1_init")
            nc.gpsimd.memset(M2[:], 0.0)
            for t in range(NSTEP):
                M1, M2 = _emit_step(nc, mybir, sb, ps, 1, t, M1, M2, cur1_1, consts, lif)
            _emit_finals(nc, mybir, psum_out, M2, w3r, 1)

            # ---- output ----
            out_sb = sb.tile([128, 32], f32, name="out_sb")
            nc.vector.tensor_add(out_sb[:], psum_out[:, 0:32], b3bc[:])
            nc.sync.dma_start(out_d[:], out_sb[:])

    nc.compile()
    return nc


def get_nc():
    if "nc" not in _CACHE:
        _CACHE["nc"] = build_nc()
    return _CACHE["nc"]


def round_fp32r(x):
    """fp32 -> fp32r encoding (RNE to 11 stored mantissa bits, low 12 bits 0).

    Matches walrus fp32_to_fp32r: the PE's fast fp32 mode reads only the top
    20 bits; pre-rounding host-side keeps the DMA a pure byte copy."""
    u = np.ascontiguousarray(np.asarray(x, np.float32)).view(np.uint32)
    keep = u & np.uint32(0xFFFFF000)
    rb = (u & np.uint32(0x800)) != 0
    sticky = ((u & np.uint32(0x7FF)) != 0) | ((u & np.uint32(0x1000)) != 0)
    up = (rb & sticky).astype(np.uint32) * np.uint32(0x1000)
    return (keep + up).view(np.float32)


def host_inputs(x, W1, b1, W2, b2, W3, b3):
    """Per-core input dicts from full inputs."""
    x = np.asarray(x, np.float32)
    W1 = np.asarray(W1, np.float32)
    b1 = np.asarray(b1, np.float32)
    W2 = np.asarray(W2, np.float32)
    b2 = np.asarray(b2, np.float32)
    W3 = np.asarray(W3, np.float32)
    b3 = np.asarray(b3, np.float32)

    xTp = np.zeros((KPAD, B_TOTAL), np.float32)
    xTp[0:KDIM, :] = x.T
    xTp = round_fp32r(xTp)
    # prepacked per-core block layout: [core][h][g][p=(b*64+k)][512j+n]
    # = xT[64*(4g+j)+k, core*2048 + 1024h + 512b + n]
    xblk = xTp.reshape(NKCH, 64, N_CORES, 2, 2, 512)  # (c,k,core,h,b,n)
    xblk = xblk.reshape(NGROUPS, 4, 64, N_CORES, 2, 2, 512)  # (g,j,k,core,h,b,n)
    xblk = np.ascontiguousarray(
        xblk.transpose(3, 4, 0, 5, 2, 1, 6)  # (core,h,g,b,k,j,n)
    ).reshape(N_CORES, 2 * NGROUPS * 128, 2048)
    w1T = W1.T  # [1700, 64]
    w1blk = np.zeros((128, NKCH * 128), np.float32)
    for c in range(NMM):
        kr = min(64, KDIM - 64 * c)
        blk = w1T[64 * c : 64 * c + kr, :]
        w1blk[0:kr, 128 * c : 128 * c + 64] = -blk
        w1blk[64 : 64 + kr, 128 * c + 64 : 128 * c + 128] = -blk
    w1blk = round_fp32r(w1blk)
    b1q = round_fp32r((-np.concatenate([b1, b1]))[None, :].astype(np.float32))
    w2p = np.zeros((128, 64), np.float32)
    w2p[0:64, 0:32] = W2.T
    w2p[64:128, 32:64] = W2.T
    w2p = round_fp32r(w2p)
    b2qn = round_fp32r((-np.tile(b2, 2))[None, :].astype(np.float32))
    w3r = round_fp32r(np.tile(np.ascontiguousarray(-W3.T), (2, 1)).astype(np.float32))
    b3bc = np.ascontiguousarray(
        np.broadcast_to(np.tile(b3, 16)[None, :], (128, 32))
    ).astype(np.float32)

    in_maps = []
    for cc in range(N_CORES):
        in_maps.append(
            {
                "xt": np.ascontiguousarray(xblk[cc]),
                "w1blk": w1blk,
                "b1q": b1q,
                "w2p": w2p,
                "b2qn": b2qn,
                "w3r": w3r,
                "b3bc": b3bc,
                "ones": np.ones((1, 512), np.float32),
                "zrow": np.zeros((1, 128), np.float32),
                "warm": np.zeros((128, 128), np.float32),
            }
        )
    return in_maps


def assemble_output(per_core_outs):
    """per_core_outs: list of [128, 32] arrays -> [16384, 2]."""
    outs = []
    for o in per_core_outs:
        o = np.asarray(o, np.float32)
        outs.append(o.reshape(128, 16, 2).transpose(1, 0, 2).reshape(BC, 2))
    return np.concatenate(outs, axis=0)


def kernel(x, W1, b1, W2, b2, W3, b3, num_steps):
    assert int(num_steps) == NSTEP, f"kernel hardcodes num_steps={NSTEP}"
    from concourse.bass_utils import run_bass_kernel_spmd

    nc = get_nc()
    in_maps = host_inputs(x, W1, b1, W2, b2, W3, b3)
    res = run_bass_kernel_spmd(nc, in_maps, core_ids=list(range(N_CORES)))
    return assemble_output([res.results[c]["out"] for c in range(N_CORES)])


# revision 24
# speedup vs baseline: 4.4832x; 1.2749x over previous
"""DeepSNN Trainium2 kernel: 2-layer LIF SNN over 20 timesteps.

Strategy (pure data parallel over 8 cores, 2048 batch rows each):
- Host pre-transposes x -> xT [1700, 2048] per core so the contraction dim
  lands on SBUF partitions with fully contiguous DMAs (no on-device transpose).
- All matmuls run in fp32r (the PE's full-rate ~12-bit-mantissa fp32 mode);
  operands are pre-rounded host-side so DMAs stay pure byte copies, and the
  on-device state writes round on write via out-AP dtype.
- fp32r matmuls require dst PSUM base partition 0, so the big matmul packs two
  batch groups per column via a K=64 block-diagonal W1 (M=128), and the cur2
  matmuls write two column ranges of one [64, 512] PSUM bank.
- Membrane state is stored NEGATED (M = -mem) so one LIF update is exactly two
  fused scalar_tensor_tensor ops:
      t  = (M * beta) - cur          (POOL engine)
      M' = (M < -1) + t              (DVE engine)
- cur2 + beta*M2 - b2 accumulate fully in PSUM (beta*I64, block-diag W2, K=1
  bias matmuls), so the mem2 update is one DVE pass reading PSUM.
- Batch is split into 2 halves per core to pipeline DMA / big matmul /
  recurrent loop; half-1's big matmul interleaves into half-0's loop on PE.
"""

import sys

if "/opt/trn_rl_repo" not in sys.path:
    sys.path.insert(0, "/opt/trn_rl_repo")

import numpy as np

B_TOTAL = 16384
N_CORES = 8
BC = 2048  # batch per core
KDIM = 1700
NKCH = 28  # K chunks of 64; chunk 26 zero-padded past row 1700, chunk 27 all pad
NMM = 27  # matmuls per half (chunk 27 is all padding, no matmul)
KPAD = 1792  # 28 * 64
HB = 1024  # batch per half
W = 512  # M1 free width per half
NSTEP = 20
BETA = 0.9

_CACHE = {}


def _register_lif_op():
    """Fused LIF update as one custom DVE op:
        out = (in0 * beta + in1) + (in0 < thresh)
    (state is negated, so this is the whole membrane update in one pass)."""
    import numpy as np
    from concourse.dve_spec import Spec, Src0, Src1, C0, C1, C2
    from concourse import dve_ops as D

    if "LIF_FUSED_SNN" in D._SUB_OPCODE_FOR_NAME:
        return D._BY_NAME_LIF
    op = D.DveOp(
        "LIF_FUSED_SNN",
        Spec(
            body=(Src0 * C1 + Src1) + (Src0 < C0),
            reference=lambda in0, in1, s0, s1, imm2: (
                (in0 * s1 + in1) + (in0 < s0)
            ).astype(np.float32),
        ),
        subdim=False,
        uops_sha={"v3": "b45eee788a1e54cc", "v4": "8b27e300884d1856"},
    )
    opb = D.DveOp(
        "LIF_FUSED_SNN_B",
        Spec(
            body=(Src0 * C1 + (Src1 + C0)) + (Src0 < C2),
            reference=lambda in0, in1, s0, s1, imm2: (
                (in0 * s1 + (in1 + s0)) + (in0 < imm2)
            ).astype(np.float32),
        ),
        subdim=False,
        uops_sha={"v3": "2a53098768c0262f", "v4": "97383be564f8766f"},
    )
    for o in (op, opb):
        D.OPS.append(o)
        D._SUB_OPCODE_FOR_NAME[o.name] = max(D._SUB_OPCODE_FOR_NAME.values()) + 1
        D.CUSTOM_DVE_SPECS[o.name] = o.spec
    D._BY_NAME_LIF = (op, opb)
    return (op, opb)


NGROUPS = 7  # groups of 4 chunks, [128, 2048] prepacked DMA blocks


def _emit_bigmm_group(nc, mybir, sb, xt_d, w1blk, psum_c1, h, g):
    """One plain [128, 2048] DMA from the host-prepacked block layout (8 KiB
    contiguous per partition -> full descriptor efficiency), then one matmul
    per real chunk. Block (h, g) row p, col 512j+n holds
    xT[64*(4g+j) + (p%64), 1024h + 512*(p//64) + n] (zero-padded past K=1700).
    """
    f16 = mybir.dt.float16
    xk = sb.tile([128, 2048], f16, tag="xk", bufs=7, name=f"xk_{h}_{g}")
    nc.sync.dma_start(xk[:], xt_d[128 * (NGROUPS * h + g) : 128 * (NGROUPS * h + g) + 128, :])
    for j in range(4):
        c = 4 * g + j
        if c >= NMM:
            continue
        nc.tensor.matmul(
            psum_c1[:, :],
            w1blk[:, 128 * c : 128 * c + 128],
            xk[:, 512 * j : 512 * j + 512],
            start=False,
            stop=False,
        )


def _emit_bigmm_half_group(nc, mybir, sb, xt_d, w1blk, psum_c1, h, idx):
    """Half a group per call: idx 0..13 -> group idx//2, mm pair idx%2."""
    f16 = mybir.dt.float16
    g, half = idx // 2, idx % 2
    if g >= NGROUPS:
        return
    if half == 0:
        xk = sb.tile([128, 2048], f16, tag="xk", bufs=7, name=f"xk_{h}_{g}")
        nc.sync.dma_start(
            xk[:], xt_d[128 * (NGROUPS * h + g) : 128 * (NGROUPS * h + g) + 128, :]
        )
        _BIGMM_TILES[(h, g)] = xk
    xk = _BIGMM_TILES[(h, g)]
    for j in (2 * half, 2 * half + 1):
        c = 4 * g + j
        if c >= NMM:
            continue
        nc.tensor.matmul(
            psum_c1[:, :],
            w1blk[:, 128 * c : 128 * c + 128],
            xk[:, 512 * j : 512 * j + 512],
            start=False,
            stop=False,
        )


_BIGMM_TILES = {}


def _emit_step(nc, mybir, sb, ps, h, t, M1, M2, cur1, consts, lif):
    f32 = mybir.dt.float32
    f32r = mybir.dt.float32r
    lif_a, lif_b = lif
    w2p, b2qn, ones = consts

    M1n = sb.tile([128, W], f32, tag=f"m1_{h}", bufs=2, name=f"m1_{h}_{t}")
    nc.vector._custom_dve(
        lif_a, out=M1n[:].bitcast(f32r), in0=M1[:], in1=cur1[:],
        s0=-1.0, s1=BETA,
    )
    pt2 = ps.tile([64, 512], f32, tag=f"t2_{h}", bufs=2, name=f"pt2_{h}_{t}")
    nc.tensor.matmul(
        pt2[:, :], w2p[:], M1n[:, :].bitcast(f32r), start=True, stop=True
    )
    M2n = sb.tile([64, 512], f32, tag=f"m2_{h}", bufs=2, name=f"m2_{h}_{t}")
    nc.vector._custom_dve(
        lif_b, out=M2n[:].bitcast(f32r), in0=M2[:], in1=pt2[:, :],
        s0=b2qn[:, 0:1], s1=BETA, imm2=-1.0,
    )
    return M1n, M2n


def _emit_finals(nc, mybir, sb, ps, M2, w3r, b3col, out_d, h):
    """outT chunk (h, a) = W3 @ m2-slice: psum [2, 512] -> ACT copy (+b3)
    -> small DMA to out_d rows 2*(2h+a)+f."""
    f32 = mybir.dt.float32
    f32r = mybir.dt.float32r
    for a in range(2):
        q = 2 * h + a
        po = ps.tile([2, 512], f32, tag="outT", bufs=2, name=f"po_{q}")
        nc.tensor.matmul(
            po[:, :],
            w3r[32 * a : 32 * a + 32, :],
            M2[32 * a : 32 * a + 32, :].bitcast(f32r),
            start=True,
            stop=True,
            tile_position=(32 * a, 0),
        )
        ot = sb.tile([2, 512], f32, tag="outT_sb", bufs=2, name=f"ot_{q}")
        nc.scalar.activation(
            ot[:], po[:, :], mybir.ActivationFunctionType.Identity,
            bias=b3col[:, 0:1], scale=1.0,
        )
        nc.sync.dma_start(out_d[2 * q : 2 * q + 2, :], ot[:])


def build_nc():
    import concourse.tile as tile
    from concourse import bacc, mybir

    lif = _register_lif_op()

    f32 = mybir.dt.float32
    f32r = mybir.dt.float32r
    f16 = mybir.dt.float16

    nc = bacc.Bacc("TRN2", target_bir_lowering=False, debug=False)
    xt_d = nc.dram_tensor("xt", [2 * NGROUPS * 128, 2048], f16, kind="ExternalInput")
    w1blk_d = nc.dram_tensor("w1blk", [128, NKCH * 128], f16, kind="ExternalInput")
    b1q_d = nc.dram_tensor("b1q", [1, 128], f32r, kind="ExternalInput")
    w2p_d = nc.dram_tensor("w2p", [128, 64], f32r, kind="ExternalInput")
    b2qn_d = nc.dram_tensor("b2qn", [64, 1], f32, kind="ExternalInput")
    w3r_d = nc.dram_tensor("w3r", [64, 2], f32r, kind="ExternalInput")
    b3col_d = nc.dram_tensor("b3col", [2, 1], f32, kind="ExternalInput")
    ones_d = nc.dram_tensor("ones", [1, 512], f32r, kind="ExternalInput")
    zrow_d = nc.dram_tensor("zrow", [1, 128], f32r, kind="ExternalInput")
    warm_d = nc.dram_tensor("warm", [128, 128], f16, kind="ExternalInput")
    out_d = nc.dram_tensor("out", [8, 512], f32, kind="ExternalOutput")

    with tile.TileContext(nc) as tc:
        with (
            tc.tile_pool(name="sb", bufs=1) as sb,
            tc.tile_pool(name="ps", bufs=1, space="PSUM") as ps,
        ):
            # ---- constants ----
            w1blk = sb.tile([128, NKCH * 128], f16, name="w1blk_sb")
            nc.sync.dma_start(w1blk[:], w1blk_d[:])
            b1q = sb.tile([1, 128], f32r, name="b1q_sb")
            nc.scalar.dma_start(b1q[:], b1q_d[:])
            w2p = sb.tile([128, 64], f32r, name="w2p_sb")
            nc.scalar.dma_start(w2p[:], w2p_d[:])
            b2qn = sb.tile([64, 1], f32, name="b2qn_sb")
            nc.scalar.dma_start(b2qn[:], b2qn_d[:])
            w3r = sb.tile([64, 2], f32r, name="w3r_sb")
            nc.scalar.dma_start(w3r[:], w3r_d[:])
            b3col = sb.tile([2, 1], f32, name="b3col_sb")
            nc.scalar.dma_start(b3col[:], b3col_d[:])
            ones = sb.tile([1, 512], f32r, name="ones_sb")
            nc.scalar.dma_start(ones[:], ones_d[:])
            zrow = sb.tile([1, 128], f32r, name="zrow_sb")
            nc.scalar.dma_start(zrow[:], zrow_d[:])
            consts = (w2p, b2qn, ones)

            # PE warmup: keep the HAM busy while the first DMAs stream in
            warm = sb.tile([128, 128], f16, name="warm_sb")
            nc.sync.dma_start(warm[:], warm_d[:])
            pc1_0 = ps.tile([128, 512], f32, tag="c1_0", bufs=1, name="pc1_0")
            for i in range(16):
                nc.tensor.matmul(
                    pc1_0[:, 0:128], warm[:], warm[:],
                    start=(i == 0), stop=(i == 15),
                )
            # bridge burst: paced by w1blk arrival, keeps HAM warm until bigmm
            for i in range(8):
                nc.tensor.matmul(
                    pc1_0[:, 0:128], w1blk[:, 0:128], warm[:],
                    start=(i == 0), stop=(i == 7),
                )

            # ---- half 0: DMA + big matmul ----
            nc.tensor.matmul(
                pc1_0[:, :], zrow[0:1, :], ones[0:1, :], start=True, stop=False
            )
            for g in range(NGROUPS):
                _emit_bigmm_group(nc, mybir, sb, xt_d, w1blk, pc1_0, 0, g)
            nc.tensor.matmul(
                pc1_0[:, :], b1q[0:1, :], ones[0:1, :], start=False, stop=True
            )
            cur1_0 = sb.tile([128, W], f32, name="cur1_0")
            nc.scalar.copy(cur1_0[:], pc1_0[:, :])

            # half-1 cur1 psum opened early so its chunks can slot in
            pc1_1 = ps.tile([128, 512], f32, tag="c1_1", bufs=1, name="pc1_1")
            nc.tensor.matmul(
                pc1_1[:, :], zrow[0:1, :], ones[0:1, :], start=True, stop=False
            )

            # ---- half 0 recurrent loop (half-1 big-mm interleaved on PE) ----
            M1 = sb.tile([128, W], f32, tag="m1_0", bufs=2, name="m1_0_init")
            nc.vector.memset(M1[:], 0.0)
            M2 = sb.tile([64, 512], f32, tag="m2_0", bufs=2, name="m2_0_init")
            nc.gpsimd.memset(M2[:], 0.0)
            for t in range(NSTEP):
                M1, M2 = _emit_step(nc, mybir, sb, ps, 0, t, M1, M2, cur1_0, consts, lif)
                if 1 <= t <= 14:
                    _emit_bigmm_half_group(
                        nc, mybir, sb, xt_d, w1blk, pc1_1, 1, t - 1
                    )
            _emit_finals(nc, mybir, sb, ps, M2, w3r, b3col, out_d, 0)

            # ---- half 1 ----
            nc.tensor.matmul(
                pc1_1[:, :], b1q[0:1, :], ones[0:1, :], start=False, stop=True
            )
            cur1_1 = sb.tile([128, W], f32, name="cur1_1")
            nc.scalar.copy(cur1_1[:], pc1_1[:, :])
            M1 = sb.tile([128, W], f32, tag="m1_1", bufs=2, name="m1_1_init")
            nc.vector.memset(M1[:], 0.0)
            M2 = sb.tile([64, 512], f32, tag="m2_1", bufs=2, name="m2_1_init")
            nc.gpsimd.memset(M2[:], 0.0)
            for t in range(NSTEP):
                M1, M2 = _emit_step(nc, mybir, sb, ps, 1, t, M1, M2, cur1_1, consts, lif)
            _emit_finals(nc, mybir, sb, ps, M2, w3r, b3col, out_d, 1)


    nc.compile()
    return nc


def get_nc():
    if "nc" not in _CACHE:
        _CACHE["nc"] = build_nc()
    return _CACHE["nc"]


def round_fp32r(x):
    """fp32 -> fp32r encoding (RNE to 11 stored mantissa bits, low 12 bits 0).

    Matches walrus fp32_to_fp32r: the PE's fast fp32 mode reads only the top
    20 bits; pre-rounding host-side keeps the DMA a pure byte copy."""
    u = np.ascontiguousarray(np.asarray(x, np.float32)).view(np.uint32)
    keep = u & np.uint32(0xFFFFF000)
    rb = (u & np.uint32(0x800)) != 0
    sticky = ((u & np.uint32(0x7FF)) != 0) | ((u & np.uint32(0x1000)) != 0)
    up = (rb & sticky).astype(np.uint32) * np.uint32(0x1000)
    return (keep + up).view(np.float32)


def host_inputs(x, W1, b1, W2, b2, W3, b3):
    """Per-core input dicts from full inputs."""
    x = np.asarray(x, np.float32)
    W1 = np.asarray(W1, np.float32)
    b1 = np.asarray(b1, np.float32)
    W2 = np.asarray(W2, np.float32)
    b2 = np.asarray(b2, np.float32)
    W3 = np.asarray(W3, np.float32)
    b3 = np.asarray(b3, np.float32)

    xTp = np.zeros((KPAD, B_TOTAL), np.float16)
    xTp[0:KDIM, :] = x.T.astype(np.float16)
    # prepacked per-core block layout: [core][h][g][p=(b*64+k)][512j+n]
    # = xT[64*(4g+j)+k, core*2048 + 1024h + 512b + n]
    xblk = xTp.reshape(NKCH, 64, N_CORES, 2, 2, 512)  # (c,k,core,h,b,n)
    xblk = xblk.reshape(NGROUPS, 4, 64, N_CORES, 2, 2, 512)  # (g,j,k,core,h,b,n)
    xblk = np.ascontiguousarray(
        xblk.transpose(3, 4, 0, 5, 2, 1, 6)  # (core,h,g,b,k,j,n)
    ).reshape(N_CORES, 2 * NGROUPS * 128, 2048)
    w1T = W1.T  # [1700, 64]
    w1blk = np.zeros((128, NKCH * 128), np.float32)
    for c in range(NMM):
        kr = min(64, KDIM - 64 * c)
        blk = w1T[64 * c : 64 * c + kr, :]
        w1blk[0:kr, 128 * c : 128 * c + 64] = -blk
        w1blk[64 : 64 + kr, 128 * c + 64 : 128 * c + 128] = -blk
    w1blk = w1blk.astype(np.float16)
    b1q = round_fp32r((-np.concatenate([b1, b1]))[None, :].astype(np.float32))
    w2p = np.zeros((128, 64), np.float32)
    w2p[0:64, 0:32] = W2.T
    w2p[64:128, 32:64] = W2.T
    w2p = round_fp32r(w2p)
    b2qn = (-np.tile(b2, 2))[:, None].astype(np.float32)
    w3r = round_fp32r(np.tile(np.ascontiguousarray(-W3.T), (2, 1)).astype(np.float32))
    b3col = b3[:, None].astype(np.float32)

    in_maps = []
    for cc in range(N_CORES):
        in_maps.append(
            {
                "xt": np.ascontiguousarray(xblk[cc]),
                "w1blk": w1blk,
                "b1q": b1q,
                "w2p": w2p,
                "b2qn": b2qn,
                "w3r": w3r,
                "b3col": b3col,
                "ones": np.ones((1, 512), np.float32),
                "zrow": np.zeros((1, 128), np.float32),
                "warm": np.zeros((128, 128), np.float16),
            }
        )
    return in_maps


def assemble_output(per_core_outs):
    """per_core_outs: list of [8, 512] arrays -> [16384, 2].

    Row 2*q+f, col n = batch 512*q + n, feat f (q = 2*h + a)."""
    outs = []
    for o in per_core_outs:
        o = np.asarray(o, np.float32)
        outs.append(o.reshape(4, 2, 512).transpose(0, 2, 1).reshape(BC, 2))
    return np.concatenate(outs, axis=0)


def kernel(x, W1, b1, W2, b2, W3, b3, num_steps):
    assert int(num_steps) == NSTEP, f"kernel hardcodes num_steps={NSTEP}"
    from concourse.bass_utils import run_bass_kernel_spmd

    nc = get_nc()
    in_maps = host_inputs(x, W1, b1, W2, b2, W3, b3)
    res = run_bass_kernel_spmd(nc, in_maps, core_ids=list(range(N_CORES)))
    return assemble_output([res.results[c]["out"] for c in range(N_CORES)])


# revision 25
# speedup vs baseline: 4.5124x; 1.0065x over previous
"""DeepSNN Trainium2 kernel: 2-layer LIF SNN over 20 timesteps.

Strategy (pure data parallel over 8 cores, 2048 batch rows each):
- Host pre-transposes x -> xT [1700, 2048] per core so the contraction dim
  lands on SBUF partitions with fully contiguous DMAs (no on-device transpose).
- All matmuls run in fp32r (the PE's full-rate ~12-bit-mantissa fp32 mode);
  operands are pre-rounded host-side so DMAs stay pure byte copies, and the
  on-device state writes round on write via out-AP dtype.
- fp32r matmuls require dst PSUM base partition 0, so the big matmul packs two
  batch groups per column via a K=64 block-diagonal W1 (M=128), and the cur2
  matmuls write two column ranges of one [64, 512] PSUM bank.
- Membrane state is stored NEGATED (M = -mem) so one LIF update is exactly two
  fused scalar_tensor_tensor ops:
      t  = (M * beta) - cur          (POOL engine)
      M' = (M < -1) + t              (DVE engine)
- cur2 + beta*M2 - b2 accumulate fully in PSUM (beta*I64, block-diag W2, K=1
  bias matmuls), so the mem2 update is one DVE pass reading PSUM.
- Batch is split into 2 halves per core to pipeline DMA / big matmul /
  recurrent loop; half-1's big matmul interleaves into half-0's loop on PE.
"""

import sys

if "/opt/trn_rl_repo" not in sys.path:
    sys.path.insert(0, "/opt/trn_rl_repo")

import numpy as np

B_TOTAL = 16384
N_CORES = 8
BC = 2048  # batch per core
KDIM = 1700
NKCH = 28  # K chunks of 64; chunk 26 zero-padded past row 1700, chunk 27 all pad
NMM = 27  # matmuls per half (chunk 27 is all padding, no matmul)
KPAD = 1792  # 28 * 64
HB = 1024  # batch per half
W = 512  # M1 free width per half
NSTEP = 20
BETA = 0.9

_CACHE = {}


def _register_lif_op():
    """Fused LIF update as one custom DVE op:
        out = (in0 * beta + in1) + (in0 < thresh)
    (state is negated, so this is the whole membrane update in one pass)."""
    import numpy as np
    from concourse.dve_spec import Spec, Src0, Src1, C0, C1, C2
    from concourse import dve_ops as D

    if "LIF_FUSED_SNN" in D._SUB_OPCODE_FOR_NAME:
        return D._BY_NAME_LIF
    op = D.DveOp(
        "LIF_FUSED_SNN",
        Spec(
            body=(Src0 * C1 + Src1) + (Src0 < C0),
            reference=lambda in0, in1, s0, s1, imm2: (
                (in0 * s1 + in1) + (in0 < s0)
            ).astype(np.float32),
        ),
        subdim=False,
        uops_sha={"v3": "b45eee788a1e54cc", "v4": "8b27e300884d1856"},
    )
    opb = D.DveOp(
        "LIF_FUSED_SNN_B",
        Spec(
            body=(Src0 * C1 + (Src1 + C0)) + (Src0 < C2),
            reference=lambda in0, in1, s0, s1, imm2: (
                (in0 * s1 + (in1 + s0)) + (in0 < imm2)
            ).astype(np.float32),
        ),
        subdim=False,
        uops_sha={"v3": "2a53098768c0262f", "v4": "97383be564f8766f"},
    )
    for o in (op, opb):
        D.OPS.append(o)
        D._SUB_OPCODE_FOR_NAME[o.name] = max(D._SUB_OPCODE_FOR_NAME.values()) + 1
        D.CUSTOM_DVE_SPECS[o.name] = o.spec
    D._BY_NAME_LIF = (op, opb)
    return (op, opb)


NGROUPS = 7  # groups of 4 chunks, [128, 2048] prepacked DMA blocks


def _emit_bigmm_group(nc, mybir, sb, xt_d, w1blk, psum_c1, h, g):  # noqa: D401
    """One plain [128, 2048] DMA from the host-prepacked block layout (8 KiB
    contiguous per partition -> full descriptor efficiency), then one matmul
    per real chunk. Block (h, g) row p, col 512j+n holds
    xT[64*(4g+j) + (p%64), 1024h + 512*(p//64) + n] (zero-padded past K=1700).
    """
    f16 = mybir.dt.float16
    xk = sb.tile([128, 2048], f16, tag="xk", bufs=7, name=f"xk_{h}_{g}")
    nc.sync.dma_start(xk[:], xt_d[128 * (NGROUPS * h + g) : 128 * (NGROUPS * h + g) + 128, :])
    for j in range(4):
        c = 4 * g + j
        if c >= NMM:
            continue
        nc.tensor.matmul(
            psum_c1[:, :],
            w1blk[:, 128 * c : 128 * c + 128],
            xk[:, 512 * j : 512 * j + 512],
            start=False,
            stop=(c == NMM - 1),
        )


def _emit_bigmm_half_group(nc, mybir, sb, xt_d, w1blk, psum_c1, h, idx):
    """Half a group per call: idx 0..13 -> group idx//2, mm pair idx%2."""
    f16 = mybir.dt.float16
    g, half = idx // 2, idx % 2
    if g >= NGROUPS:
        return
    if half == 0:
        xk = sb.tile([128, 2048], f16, tag="xk", bufs=7, name=f"xk_{h}_{g}")
        nc.sync.dma_start(
            xk[:], xt_d[128 * (NGROUPS * h + g) : 128 * (NGROUPS * h + g) + 128, :]
        )
        _BIGMM_TILES[(h, g)] = xk
    xk = _BIGMM_TILES[(h, g)]
    for j in (2 * half, 2 * half + 1):
        c = 4 * g + j
        if c >= NMM:
            continue
        nc.tensor.matmul(
            psum_c1[:, :],
            w1blk[:, 128 * c : 128 * c + 128],
            xk[:, 512 * j : 512 * j + 512],
            start=False,
            stop=(c == NMM - 1),
        )


_BIGMM_TILES = {}


def _emit_step(nc, mybir, sb, ps, h, t, M1, M2, cur1, consts, lif):
    f32 = mybir.dt.float32
    f32r = mybir.dt.float32r
    lif_a, lif_b = lif
    w2p, b2qn, ones = consts

    M1n = sb.tile([128, W], f32, tag=f"m1_{h}", bufs=2, name=f"m1_{h}_{t}")
    nc.vector._custom_dve(
        lif_a, out=M1n[:].bitcast(f32r), in0=M1[:], in1=cur1[:],
        s0=-1.0, s1=BETA,
    )
    pt2 = ps.tile([64, 512], f32, tag=f"t2_{h}", bufs=2, name=f"pt2_{h}_{t}")
    nc.tensor.matmul(
        pt2[:, :], w2p[:], M1n[:, :].bitcast(f32r), start=True, stop=True
    )
    M2n = sb.tile([64, 512], f32, tag=f"m2_{h}", bufs=2, name=f"m2_{h}_{t}")
    nc.vector._custom_dve(
        lif_b, out=M2n[:].bitcast(f32r), in0=M2[:], in1=pt2[:, :],
        s0=b2qn[:, 0:1], s1=BETA, imm2=-1.0,
    )
    return M1n, M2n


def _emit_finals(nc, mybir, sb, ps, M2, w3r, b3col, out_d, h):
    """outT chunk (h, a) = W3 @ m2-slice: psum [2, 512] -> ACT copy (+b3)
    -> small DMA to out_d rows 2*(2h+a)+f."""
    f32 = mybir.dt.float32
    f32r = mybir.dt.float32r
    for a in range(2):
        q = 2 * h + a
        po = ps.tile([2, 512], f32, tag="outT", bufs=2, name=f"po_{q}")
        nc.tensor.matmul(
            po[:, :],
            w3r[32 * a : 32 * a + 32, :],
            M2[32 * a : 32 * a + 32, :].bitcast(f32r),
            start=True,
            stop=True,
            tile_position=(32 * a, 0),
        )
        ot = sb.tile([2, 512], f32, tag="outT_sb", bufs=2, name=f"ot_{q}")
        nc.scalar.activation(
            ot[:], po[:, :], mybir.ActivationFunctionType.Identity,
            bias=b3col[:, 0:1], scale=1.0,
        )
        nc.sync.dma_start(out_d[2 * q : 2 * q + 2, :], ot[:])


def build_nc():
    import concourse.tile as tile
    from concourse import bacc, mybir

    lif = _register_lif_op()

    f32 = mybir.dt.float32
    f32r = mybir.dt.float32r
    f16 = mybir.dt.float16

    nc = bacc.Bacc("TRN2", target_bir_lowering=False, debug=False)
    xt_d = nc.dram_tensor("xt", [2 * NGROUPS * 128, 2048], f16, kind="ExternalInput")
    w1blk_d = nc.dram_tensor("w1blk", [128, NKCH * 128], f16, kind="ExternalInput")
    b1q_d = nc.dram_tensor("b1q", [1, 128], f32r, kind="ExternalInput")
    w2p_d = nc.dram_tensor("w2p", [128, 64], f32r, kind="ExternalInput")
    b2qn_d = nc.dram_tensor("b2qn", [64, 1], f32, kind="ExternalInput")
    w3r_d = nc.dram_tensor("w3r", [64, 2], f32r, kind="ExternalInput")
    b3col_d = nc.dram_tensor("b3col", [2, 1], f32, kind="ExternalInput")
    ones_d = nc.dram_tensor("ones", [1, 512], f32r, kind="ExternalInput")
    warm_d = nc.dram_tensor("warm", [128, 128], f16, kind="ExternalInput")
    out_d = nc.dram_tensor("out", [8, 512], f32, kind="ExternalOutput")

    with tile.TileContext(nc) as tc:
        with (
            tc.tile_pool(name="sb", bufs=1) as sb,
            tc.tile_pool(name="ps", bufs=1, space="PSUM") as ps,
        ):
            # ---- constants ----
            w1blk = sb.tile([128, NKCH * 128], f16, name="w1blk_sb")
            nc.sync.dma_start(w1blk[:], w1blk_d[:])
            b1q = sb.tile([1, 128], f32r, name="b1q_sb")
            nc.scalar.dma_start(b1q[:], b1q_d[:])
            w2p = sb.tile([128, 64], f32r, name="w2p_sb")
            nc.scalar.dma_start(w2p[:], w2p_d[:])
            b2qn = sb.tile([64, 1], f32, name="b2qn_sb")
            nc.scalar.dma_start(b2qn[:], b2qn_d[:])
            w3r = sb.tile([64, 2], f32r, name="w3r_sb")
            nc.scalar.dma_start(w3r[:], w3r_d[:])
            b3col = sb.tile([2, 1], f32, name="b3col_sb")
            nc.scalar.dma_start(b3col[:], b3col_d[:])
            ones = sb.tile([1, 512], f32r, name="ones_sb")
            nc.scalar.dma_start(ones[:], ones_d[:])
            consts = (w2p, b2qn, ones)

            # PE warmup: keep the HAM busy while the first DMAs stream in
            warm = sb.tile([128, 128], f16, name="warm_sb")
            nc.sync.dma_start(warm[:], warm_d[:])
            pc1_0 = ps.tile([128, 512], f32, tag="c1_0", bufs=1, name="pc1_0")
            for i in range(16):
                nc.tensor.matmul(
                    pc1_0[:, 0:128], warm[:], warm[:],
                    start=(i == 0), stop=(i == 15),
                )
            # bridge burst: paced by w1blk arrival, keeps HAM warm until bigmm
            for i in range(8):
                nc.tensor.matmul(
                    pc1_0[:, 0:128], w1blk[:, 0:128], warm[:],
                    start=(i == 0), stop=(i == 7),
                )

            # ---- half 0: DMA + big matmul ----
            nc.tensor.matmul(
                pc1_0[:, :], b1q[0:1, :], ones[0:1, :], start=True, stop=False
            )
            for g in range(NGROUPS):
                _emit_bigmm_group(nc, mybir, sb, xt_d, w1blk, pc1_0, 0, g)
            cur1_0 = sb.tile([128, W], f32, name="cur1_0")
            nc.scalar.copy(cur1_0[:], pc1_0[:, :])

            # half-1 cur1 psum opened early so its chunks can slot in
            pc1_1 = ps.tile([128, 512], f32, tag="c1_1", bufs=1, name="pc1_1")
            nc.tensor.matmul(
                pc1_1[:, :], b1q[0:1, :], ones[0:1, :], start=True, stop=False
            )

            # ---- half 0 recurrent loop (half-1 big-mm interleaved on PE) ----
            M1 = sb.tile([128, W], f32, tag="m1_0", bufs=2, name="m1_0_init")
            nc.vector.memset(M1[:], 0.0)
            M2 = sb.tile([64, 512], f32, tag="m2_0", bufs=2, name="m2_0_init")
            nc.gpsimd.memset(M2[:], 0.0)
            for t in range(NSTEP):
                M1, M2 = _emit_step(nc, mybir, sb, ps, 0, t, M1, M2, cur1_0, consts, lif)
                if 1 <= t <= 14:
                    _emit_bigmm_half_group(
                        nc, mybir, sb, xt_d, w1blk, pc1_1, 1, t - 1
                    )
            _emit_finals(nc, mybir, sb, ps, M2, w3r, b3col, out_d, 0)

            # ---- half 1 ----
            cur1_1 = sb.tile([128, W], f32, name="cur1_1")
            nc.scalar.copy(cur1_1[:], pc1_1[:, :])
            M1 = sb.tile([128, W], f32, tag="m1_1", bufs=2, name="m1_1_init")
            nc.vector.memset(M1[:], 0.0)
            M2 = sb.tile([64, 512], f32, tag="m2_1", bufs=2, name="m2_1_init")
            nc.gpsimd.memset(M2[:], 0.0)
            for t in range(NSTEP):
                M1, M2 = _emit_step(nc, mybir, sb, ps, 1, t, M1, M2, cur1_1, consts, lif)
            _emit_finals(nc, mybir, sb, ps, M2, w3r, b3col, out_d, 1)


    nc.compile()
    return nc


def get_nc():
    if "nc" not in _CACHE:
        _CACHE["nc"] = build_nc()
    return _CACHE["nc"]


def round_fp32r(x):
    """fp32 -> fp32r encoding (RNE to 11 stored mantissa bits, low 12 bits 0).

    Matches walrus fp32_to_fp32r: the PE's fast fp32 mode reads only the top
    20 bits; pre-rounding host-side keeps the DMA a pure byte copy."""
    u = np.ascontiguousarray(np.asarray(x, np.float32)).view(np.uint32)
    keep = u & np.uint32(0xFFFFF000)
    rb = (u & np.uint32(0x800)) != 0
    sticky = ((u & np.uint32(0x7FF)) != 0) | ((u & np.uint32(0x1000)) != 0)
    up = (rb & sticky).astype(np.uint32) * np.uint32(0x1000)
    return (keep + up).view(np.float32)


def host_inputs(x, W1, b1, W2, b2, W3, b3):
    """Per-core input dicts from full inputs."""
    x = np.asarray(x, np.float32)
    W1 = np.asarray(W1, np.float32)
    b1 = np.asarray(b1, np.float32)
    W2 = np.asarray(W2, np.float32)
    b2 = np.asarray(b2, np.float32)
    W3 = np.asarray(W3, np.float32)
    b3 = np.asarray(b3, np.float32)

    xTp = np.zeros((KPAD, B_TOTAL), np.float16)
    xTp[0:KDIM, :] = x.T.astype(np.float16)
    # prepacked per-core block layout: [core][h][g][p=(b*64+k)][512j+n]
    # = xT[64*(4g+j)+k, core*2048 + 1024h + 512b + n]
    xblk = xTp.reshape(NKCH, 64, N_CORES, 2, 2, 512)  # (c,k,core,h,b,n)
    xblk = xblk.reshape(NGROUPS, 4, 64, N_CORES, 2, 2, 512)  # (g,j,k,core,h,b,n)
    xblk = np.ascontiguousarray(
        xblk.transpose(3, 4, 0, 5, 2, 1, 6)  # (core,h,g,b,k,j,n)
    ).reshape(N_CORES, 2 * NGROUPS * 128, 2048)
    w1T = W1.T  # [1700, 64]
    w1blk = np.zeros((128, NKCH * 128), np.float32)
    for c in range(NMM):
        kr = min(64, KDIM - 64 * c)
        blk = w1T[64 * c : 64 * c + kr, :]
        w1blk[0:kr, 128 * c : 128 * c + 64] = -blk
        w1blk[64 : 64 + kr, 128 * c + 64 : 128 * c + 128] = -blk
    w1blk = w1blk.astype(np.float16)
    b1q = round_fp32r((-np.concatenate([b1, b1]))[None, :].astype(np.float32))
    w2p = np.zeros((128, 64), np.float32)
    w2p[0:64, 0:32] = W2.T
    w2p[64:128, 32:64] = W2.T
    w2p = round_fp32r(w2p)
    b2qn = (-np.tile(b2, 2))[:, None].astype(np.float32)
    w3r = round_fp32r(np.tile(np.ascontiguousarray(-W3.T), (2, 1)).astype(np.float32))
    b3col = b3[:, None].astype(np.float32)

    in_maps = []
    for cc in range(N_CORES):
        in_maps.append(
            {
                "xt": np.ascontiguousarray(xblk[cc]),
                "w1blk": w1blk,
                "b1q": b1q,
                "w2p": w2p,
                "b2qn": b2qn,
                "w3r": w3r,
                "b3col": b3col,
                "ones": np.ones((1, 512), np.float32),
                "warm": np.zeros((128, 128), np.float16),
            }
        )
    return in_maps


def assemble_output(per_core_outs):
    """per_core_outs: list of [8, 512] arrays -> [16384, 2].

    Row 2*q+f, col n = batch 512*q + n, feat f (q = 2*h + a)."""
    outs = []
    for o in per_core_outs:
        o = np.asarray(o, np.float32)
        outs.append(o.reshape(4, 2, 512).transpose(0, 2, 1).reshape(BC, 2))
    return np.concatenate(outs, axis=0)


def kernel(x, W1, b1, W2, b2, W3, b3, num_steps):
    assert int(num_steps) == NSTEP, f"kernel hardcodes num_steps={NSTEP}"
    from concourse.bass_utils import run_bass_kernel_spmd

    nc = get_nc()
    in_maps = host_inputs(x, W1, b1, W2, b2, W3, b3)
    res = run_bass_kernel_spmd(nc, in_maps, core_ids=list(range(N_CORES)))
    return assemble_output([res.results[c]["out"] for c in range(N_CORES)])
